# revision 4
# baseline (speedup 1.0000x reference)
"""Trainium2 Bass kernel for nn_DGC (deep graph clustering GNN).

Full inputs in, full outputs out. Internally: row-shards the node dimension
N=10000 (padded to 10240) across 8 NeuronCores; adj is passed per-core as a
transposed (k-node major) block so the dense spmm adj @ (h @ W) maps directly
onto the PE array; per-layer activations (h @ W) are AllGathered across cores.

Dataflow (per core, zero on-device transposes):
  - activations kept feature-major [E, 1280]; AE chain = relu(W.T @ x.T)
  - GNN residual mix folded into host-scaled weights: stored tra' = (3/7)*tra,
    gnn_w' = 0.7*gnn_w, so u = h + tra' with a single DVE add
  - xw computed node-major via lhsT=u.T-tile, rhs=W -> AllGather (axis 0)
  - spmm: lhsT = xw_full k-tile (stationary), rhs = adjT k-tile (moving),
    accumulated over 80 k-tiles into PSUM -> h.T feature-major
  - q head: d2 via augmented matmul [-2c.T; |c|^2] with ones row; predict via
    PE-transpose + free-dim softmax. Compute bf16, accumulate/heads f32.
"""

import numpy as np
import ml_dtypes

BF16 = ml_dtypes.bfloat16

NCORES = 8
N = 10000
NP = 10240
R = NP // NCORES          # 1280 rows per core
MT = R // 128             # 10 m-tiles
KT = NP // 128            # 80 k-tiles (spmm contraction)
NI, E1, E2, E3, NZ, K = 1024, 512, 512, 2048, 64, 16
SIGMA = 0.3
TS = SIGMA / (1.0 - SIGMA)   # tra storage scale (3/7)
MCH = [(0, 512), (512, 512), (1024, 256)]  # m-chunks of the 1280 local nodes
L3_SLABS = [(0, 768), (768, 768), (1536, 512)]

_cached_nc = None


def _build():
    import concourse.bacc as bacc
    import concourse.mybir as mybir
    import concourse.tile as tile
    from concourse.masks import make_identity
    from contextlib import ExitStack

    dt = mybir.dt
    AF = mybir.ActivationFunctionType
    AX = mybir.AxisListType

    nc = bacc.Bacc("TRN2", target_bir_lowering=False, debug=False,
                   num_devices=NCORES)

    # ---- I/O ----
    f32, bf = dt.float32, dt.bfloat16
    xT_d = nc.dram_tensor("xT", [NI, R], bf, kind="ExternalInput")
    adjT_d = nc.dram_tensor("adjT", [NP, R], bf, kind="ExternalInput")
    e1w_d = nc.dram_tensor("e1w", [NI, E1], bf, kind="ExternalInput")
    e2w_d = nc.dram_tensor("e2w", [E1, E2], bf, kind="ExternalInput")
    e3w_d = nc.dram_tensor("e3w", [E2, E3], bf, kind="ExternalInput")
    zw_d = nc.dram_tensor("zw", [E3, NZ], bf, kind="ExternalInput")
    e1b_d = nc.dram_tensor("e1b", [E1, 1], f32, kind="ExternalInput")
    e2b_d = nc.dram_tensor("e2b", [E2, 1], f32, kind="ExternalInput")
    e3b_d = nc.dram_tensor("e3b", [E3, 1], f32, kind="ExternalInput")
    zb_d = nc.dram_tensor("zb", [NZ, 1], f32, kind="ExternalInput")
    g1w_d = nc.dram_tensor("g1w", [NI, E1], bf, kind="ExternalInput")
    g2w_d = nc.dram_tensor("g2w", [E1, E2], bf, kind="ExternalInput")
    g3w_d = nc.dram_tensor("g3w", [E2, E3], bf, kind="ExternalInput")
    g4w_d = nc.dram_tensor("g4w", [E3, NZ], bf, kind="ExternalInput")
    g5wp_d = nc.dram_tensor("g5wp", [128, K], bf, kind="ExternalInput")
    fc1aug_d = nc.dram_tensor("fc1aug", [128, NI], bf, kind="ExternalInput")
    qmat_d = nc.dram_tensor("qmat", [128, K], f32, kind="ExternalInput")

    xbar_o = nc.dram_tensor("xbar_o", [R, NI], f32, kind="ExternalOutput")
    q_o = nc.dram_tensor("q_o", [R, K], f32, kind="ExternalOutput")
    pred_o = nc.dram_tensor("pred_o", [R, K], f32, kind="ExternalOutput")
    z_o = nc.dram_tensor("z_o", [NZ, R], f32, kind="ExternalOutput")

    rg = [list(range(NCORES))]

    with tile.TileContext(nc) as tc:
        with ExitStack() as ctx:
            dram = ctx.enter_context(tc.tile_pool(name="dram", bufs=1, space="DRAM"))
            stage = ctx.enter_context(tc.tile_pool(name="stage", bufs=3))
            small = ctx.enter_context(tc.tile_pool(name="small", bufs=4))
            dpsum = ctx.enter_context(tc.tile_pool(name="dpsum", bufs=2, space="PSUM"))
            main = ctx.enter_context(tc.tile_pool(name="main", bufs=1))

            # ---- DRAM scratch ----
            def ag_pair(name, e):
                loc = dram.tile([R, e], bf, name=f"{name}_loc")
                full = dram.tile([NP, e], bf, name=f"{name}_full",
                                 addr_space="Shared")
                return loc, full

            xw1_loc, xw1_full = ag_pair("xw1", E1)
            xw2_loc, xw2_full = ag_pair("xw2", E2)
            xw3_loc, xw3_full = ag_pair("xw3", E3)
            xw4_loc, xw4_full = ag_pair("xw4", NZ)
            xw5_loc, xw5_full = ag_pair("xw5", K)
            tra3_dram = dram.tile([128, E3 // 128, R], bf, name="tra3_dram")
            h3_dram = dram.tile([128, E3 // 128, R], bf, name="h3_dram")

            # ---- long-lived small SBUF (main pool) ----
            z_sb = main.tile([NZ, R], bf, name="z_sb")
            h4aug = main.tile([128, R], f32, name="h4aug")
            rh4_sb = main.tile([128, R], bf, name="rh4_sb")
            h5_sb = main.tile([16, R], f32, name="h5_sb")
            h4sq = main.tile([128, R], f32, name="h4sq")
            qmat_sb = main.tile([128, K], f32, name="qmat_sb")
            ones_sb = main.tile([128, K], f32, name="ones_sb")
            ident_sb = main.tile([16, 16], f32, name="ident_sb")

            nc.gpsimd.memset(h4aug[:], 0.0)
            nc.gpsimd.memset(h4aug[NZ:NZ + 1, :], 1.0)   # ones row for q matmul
            nc.gpsimd.memset(rh4_sb[:], 0.0)
            nc.gpsimd.memset(rh4_sb[NZ:NZ + 1, :], 1.0)  # ones row for fc1 bias
            nc.sync.dma_start(qmat_sb[:], qmat_d[:])
            nc.gpsimd.memset(ones_sb[:], 0.0)
            nc.gpsimd.memset(ones_sb[:NZ, :], 1.0)
            make_identity(nc, ident_sb[:])

            # ---- helpers ----
            def load_w(pool, w_d, kf_dim, eo):
                w = pool.tile([128, kf_dim // 128, eo], bf, name=f"w_{w_d.name}")
                nc.sync.dma_start(w[:], w_d.ap().rearrange("(o p) e -> p o e", p=128))
                return w

            def load_b(b_d, eo):
                nchunks = max(1, eo // 128)
                p = min(128, eo)
                b = small.tile([p, nchunks], f32, name=f"b_{b_d.name}", tag="bias")
                nc.sync.dma_start(b[:], b_d.ap().rearrange("(o p) x -> p (o x)", p=p))
                return b

            def dense_fm(in_sb, w_d, b_d, kf, eo, drain):
                """Feature-major dense: psum[n_chunk, m] = W.T @ in.T."""
                with tc.tile_pool(name=f"wp_{w_d.name}", bufs=1) as wp:
                    w_sb = load_w(wp, w_d, kf, eo)
                    b_sb = load_b(b_d, eo) if b_d is not None else None
                    for nci in range(eo // 128):
                        for (mo, mw) in MCH:
                            ps = dpsum.tile([128, 512], f32, tag="dps",
                                            name=f"dps_{w_d.name}_{nci}_{mo}")
                            for kfi in range(kf // 128):
                                nc.tensor.matmul(
                                    ps[:, :mw],
                                    lhsT=w_sb[:, kfi, nci * 128:(nci + 1) * 128],
                                    rhs=in_sb[:, kfi, mo:mo + mw],
                                    start=(kfi == 0), stop=(kfi == kf // 128 - 1))
                            bias_ap = b_sb[:, nci:nci + 1] if b_sb is not None else None
                            drain(nci, mo, mw, ps[:, :mw], bias_ap)

            def dense_nm(u_sb, w_sb, kf, eo, mt_drain, out_dt=bf):
                """Node-major dense: psum[m_tile, n] = u @ W; mt_drain per chunk."""
                for mt in range(MT):
                    st = stage.tile([128, max(eo, 128)], out_dt, tag="nmstage",
                                    name=f"nmst_{mt}")
                    for nco in range((eo + 511) // 512):
                        ncw = min(512, eo - nco * 512)
                        ps = dpsum.tile([128, 512], f32, tag="dps",
                                        name=f"nmps_{mt}_{nco}")
                        nkf = max(1, kf // 128)
                        for kfi in range(nkf):
                            lhs = u_sb[:, kfi, mt * 128:(mt + 1) * 128] if kf > 128 \
                                else u_sb[:, mt * 128:(mt + 1) * 128]
                            rhs = w_sb[:, kfi, nco * 512:nco * 512 + ncw] if kf > 128 \
                                else w_sb[:, nco * 512:nco * 512 + ncw]
                            nc.tensor.matmul(ps[:, :ncw], lhsT=lhs, rhs=rhs,
                                             start=(kfi == 0), stop=(kfi == nkf - 1))
                        mt_drain(mt, nco, ncw, ps[:, :ncw], st)

            def all_gather(loc, full):
                nc.gpsimd.collective_compute(
                    "AllGather", mybir.AluOpType.bypass, replica_groups=rg,
                    ins=[loc.opt()], outs=[full.opt()])

            def spmm(xw_full, e, slabs, drain):
                """h.T[n, m] = sum_k xw_full[k, n] * adjT[k, m]."""
                adjT_r = adjT_d.ap().rearrange("(o p) m -> p o m", p=128)
                with (
                    tc.tile_pool(name=f"slab_{e}", bufs=1) as slp,
                    tc.tile_pool(name=f"adjp_{e}", bufs=3) as adp,
                    tc.tile_pool(name=f"psp_{e}", bufs=1, space="PSUM") as pp,
                ):
                    for (soff, scols) in slabs:
                        slab = slp.tile([128, KT, scols], bf, tag="slab",
                                        name=f"slab_{e}_{soff}")
                        for ko in range(KT):
                            nc.sync.dma_start(
                                slab[:, ko, :],
                                xw_full[ko * 128:(ko + 1) * 128, soff:soff + scols])
                        G = max(1, scols // 128)
                        gw = min(128, scols)
                        for (mo, mw) in MCH:
                            pss = [pp.tile([128, 512], f32, tag=f"ps{g}",
                                           name=f"ps_{e}_{soff}_{mo}_{g}")[:gw, :mw]
                                   for g in range(G)]
                            for ko in range(0, KT, 2):
                                adj2 = adp.tile([128, 2, 512], bf, tag="adj",
                                                name=f"adj_{e}_{soff}_{mo}_{ko}")
                                nc.sync.dma_start(
                                    adj2[:, :, :mw],
                                    adjT_r[:, ko:ko + 2, mo:mo + mw])
                                for kk in range(2):
                                    k = ko + kk
                                    for g in range(G):
                                        nc.tensor.matmul(
                                            pss[g],
                                            lhsT=slab[:, k, g * gw:(g + 1) * gw],
                                            rhs=adj2[:, kk, :mw],
                                            start=(k == 0), stop=(k == KT - 1))
                            for g in range(G):
                                drain(soff // 128 + g, mo, mw, pss[g])

            # ================= Program =================

            with ExitStack() as phaseA:
                pA = phaseA.enter_context(tc.tile_pool(name="pA", bufs=1))
                tra1_sb = pA.tile([128, E1 // 128, R], bf, name="tra1_sb")
                tra2_sb = pA.tile([128, E2 // 128, R], bf, name="tra2_sb")
                h1_sb = pA.tile([128, E1 // 128, R], bf, name="h1_sb")
                h2_sb = pA.tile([128, E2 // 128, R], bf, name="h2_sb")

                # -- xw1 = x @ g1w (node-major) -> AG; AE enc1 (both need xT) --
                with tc.tile_pool(name="pX", bufs=1) as pX:
                    xT_sb = pX.tile([128, NI // 128, R], bf, name="xT_sb")
                    nc.sync.dma_start(
                        xT_sb[:], xT_d.ap().rearrange("(o p) m -> p o m", p=128))

                    with tc.tile_pool(name="wg1", bufs=1) as wp:
                        g1w_sb = load_w(wp, g1w_d, NI, E1)

                        def xw1_drain(mt, nco, ncw, ps, st):
                            nc.scalar.copy(st[:, nco * 512:nco * 512 + ncw], ps)
                            if nco == (E1 + 511) // 512 - 1:
                                nc.sync.dma_start(
                                    xw1_loc[mt * 128:(mt + 1) * 128, :], st[:, :E1])

                        dense_nm(xT_sb, g1w_sb, NI, E1, xw1_drain)
                    all_gather(xw1_loc, xw1_full)

                    def fm_relu_drain(out_sb):
                        def d(nci, mo, mw, ps, b):
                            nc.scalar.activation(out_sb[:, nci, mo:mo + mw], ps,
                                                 AF.Relu,
                                                 bias=b if b is not None else 0.0)
                        return d

                    dense_fm(xT_sb, e1w_d, e1b_d, NI, E1, fm_relu_drain(tra1_sb))

                # -- AE enc2/enc3 (tra3' straight to DRAM) --
                dense_fm(tra1_sb, e2w_d, e2b_d, E1, E2, fm_relu_drain(tra2_sb))

                def tra3_drain(nci, mo, mw, ps, b):
                    st = stage.tile([128, 512], bf, tag="fmstage",
                                    name=f"t3st_{nci}_{mo}")
                    nc.scalar.activation(st[:, :mw], ps, AF.Relu,
                                         bias=b if b is not None else 0.0)
                    nc.sync.dma_start(tra3_dram[:, nci, mo:mo + mw], st[:, :mw])

                dense_fm(tra2_sb, e3w_d, e3b_d, E2, E3, tra3_drain)

                # -- z' = tra3' @ zw + zb' (stream tra3' back from DRAM) --
                with tc.tile_pool(name="wz", bufs=1) as wp, \
                     tc.tile_pool(name="t3s", bufs=3) as t3p:
                    zw_sb = load_w(wp, zw_d, E3, NZ)
                    zb_sb = load_b(zb_d, NZ)
                    for (mo, mw) in MCH:
                        ps = dpsum.tile([128, 512], f32, tag="dps", name=f"zps_{mo}")
                        for kfi in range(E3 // 128):
                            t3t = t3p.tile([128, 512], bf, tag="t3t",
                                           name=f"t3t_{mo}_{kfi}")
                            nc.sync.dma_start(t3t[:, :mw],
                                              tra3_dram[:, kfi, mo:mo + mw])
                            nc.tensor.matmul(ps[:NZ, :mw], lhsT=zw_sb[:, kfi, :NZ],
                                             rhs=t3t[:, :mw], start=(kfi == 0),
                                             stop=(kfi == E3 // 128 - 1))
                        nc.scalar.activation(z_sb[:, mo:mo + mw], ps[:NZ, :mw],
                                             AF.Identity, bias=zb_sb[:, 0:1])
                        zst = stage.tile([NZ, 512], f32, tag="zstage",
                                         name=f"zst_{mo}")
                        nc.scalar.activation(zst[:, :mw], ps[:NZ, :mw], AF.Identity,
                                             bias=zb_sb[:, 0:1])
                        nc.scalar.mul(zst[:, :mw], zst[:, :mw], 1.0 / TS)
                        nc.sync.dma_start(z_o[:, mo:mo + mw], zst[:, :mw])

                # -- spmm1: h1 = relu(adj @ xw1) --
                def h_drain(out_sb):
                    def d(gg, mo, mw, ps):
                        nc.scalar.activation(out_sb[:, gg, mo:mo + mw], ps, AF.Relu)
                    return d

                spmm(xw1_full, E1, [(0, E1)], h_drain(h1_sb))

                # -- u2 = h1 + tra1'; xw2 -> AG --
                for kfi in range(E1 // 128):
                    nc.vector.tensor_add(out=h1_sb[:, kfi, :],
                                         in0=h1_sb[:, kfi, :],
                                         in1=tra1_sb[:, kfi, :])
                with tc.tile_pool(name="wg2", bufs=1) as wp:
                    g2w_sb = load_w(wp, g2w_d, E1, E2)

                    def xw2_drain(mt, nco, ncw, ps, st):
                        nc.scalar.copy(st[:, nco * 512:nco * 512 + ncw], ps)
                        if nco == (E2 + 511) // 512 - 1:
                            nc.sync.dma_start(
                                xw2_loc[mt * 128:(mt + 1) * 128, :], st[:, :E2])

                    dense_nm(h1_sb, g2w_sb, E1, E2, xw2_drain)
                all_gather(xw2_loc, xw2_full)

                # -- spmm2: h2 --
                spmm(xw2_full, E2, [(0, E2)], h_drain(h2_sb))

                # -- u3 = h2 + tra2'; xw3 -> AG --
                for kfi in range(E2 // 128):
                    nc.vector.tensor_add(out=h2_sb[:, kfi, :],
                                         in0=h2_sb[:, kfi, :],
                                         in1=tra2_sb[:, kfi, :])
                with tc.tile_pool(name="wg3", bufs=1) as wp:
                    g3w_sb = load_w(wp, g3w_d, E2, E3)

                    def xw3_drain(mt, nco, ncw, ps, st):
                        nc.scalar.copy(st[:, nco * 512:nco * 512 + ncw], ps)
                        if nco == (E3 + 511) // 512 - 1:
                            nc.sync.dma_start(
                                xw3_loc[mt * 128:(mt + 1) * 128, :], st[:, :E3])

                    dense_nm(h2_sb, g3w_sb, E2, E3, xw3_drain)
                all_gather(xw3_loc, xw3_full)

            # -- spmm3: h3 -> DRAM (phase-A SBUF is freed) --
            def h3_drain(gg, mo, mw, ps):
                st = stage.tile([128, 512], bf, tag="fmstage",
                                name=f"h3st_{gg}_{mo}")
                nc.scalar.activation(st[:, :mw], ps, AF.Relu)
                nc.sync.dma_start(h3_dram[:, gg, mo:mo + mw], st[:, :mw])

            spmm(xw3_full, E3, L3_SLABS, h3_drain)

            # -- u4 = h3 + tra3'; xw4 -> AG --
            with ExitStack() as phaseB:
                pB = phaseB.enter_context(tc.tile_pool(name="pB", bufs=1))
                u4_sb = pB.tile([128, E3 // 128, R], bf, name="u4_sb")
                with tc.tile_pool(name="u4l", bufs=4) as up:
                    for kfi in range(E3 // 128):
                        a = up.tile([128, R], bf, tag="u4a", name=f"u4a_{kfi}")
                        b = up.tile([128, R], bf, tag="u4b", name=f"u4b_{kfi}")
                        nc.sync.dma_start(a[:], h3_dram[:, kfi, :])
                        nc.sync.dma_start(b[:], tra3_dram[:, kfi, :])
                        nc.vector.tensor_add(out=u4_sb[:, kfi, :], in0=a[:],
                                             in1=b[:])
                with tc.tile_pool(name="wg4", bufs=1) as wp:
                    g4w_sb = load_w(wp, g4w_d, E3, NZ)

                    def xw4_drain(mt, nco, ncw, ps, st):
                        nc.scalar.copy(st[:, :ncw], ps)
                        nc.sync.dma_start(xw4_loc[mt * 128:(mt + 1) * 128, :],
                                          st[:, :NZ])

                    dense_nm(u4_sb, g4w_sb, E3, NZ, xw4_drain)
                all_gather(xw4_loc, xw4_full)

            # -- spmm4: h4 (no relu) -> h4aug f32 + relu(h4) bf16 --
            def h4_drain(gg, mo, mw, ps):
                nc.vector.tensor_copy(out=h4aug[:NZ, mo:mo + mw], in_=ps)
                nc.scalar.activation(rh4_sb[:NZ, mo:mo + mw], ps, AF.Relu)

            spmm(xw4_full, NZ, [(0, NZ)], h4_drain)

            # -- x_bar = relu(relu(h4) @ fc1_w + b) (bias via ones-row aug) --
            with tc.tile_pool(name="wfc", bufs=1) as wp:
                fc1_sb = wp.tile([128, NI], bf, name="fc1_sb")
                nc.sync.dma_start(fc1_sb[:], fc1aug_d[:])

                def fc1_drain(mt, nco, ncw, ps, st):
                    nc.scalar.activation(st[:, nco * 512:nco * 512 + ncw], ps,
                                         AF.Relu)
                    if nco == (NI + 511) // 512 - 1:
                        nc.sync.dma_start(xbar_o[mt * 128:(mt + 1) * 128, :],
                                          st[:, :NI])

                dense_nm(rh4_sb, fc1_sb, 128, NI, fc1_drain, out_dt=f32)

            # -- u5 = relu(h4) + z'; xw5 -> AG --
            with ExitStack() as phaseC:
                pC = phaseC.enter_context(tc.tile_pool(name="pC", bufs=1))
                u5_sb = pC.tile([128, R], bf, name="u5_sb")
                nc.gpsimd.memset(u5_sb[:], 0.0)
                nc.vector.tensor_add(out=u5_sb[:NZ, :], in0=rh4_sb[:NZ, :],
                                     in1=z_sb[:])
                with tc.tile_pool(name="wg5", bufs=1) as wp:
                    g5w_sb = wp.tile([128, K], bf, name="g5w_sb")
                    nc.sync.dma_start(g5w_sb[:], g5wp_d[:])

                    def xw5_drain(mt, nco, ncw, ps, st):
                        nc.scalar.copy(st[:, :ncw], ps)
                        nc.sync.dma_start(xw5_loc[mt * 128:(mt + 1) * 128, :],
                                          st[:, :K])

                    dense_nm(u5_sb, g5w_sb, 128, K, xw5_drain)
                all_gather(xw5_loc, xw5_full)

            # -- spmm5: h5 (no relu) -> f32 --
            def h5_drain(gg, mo, mw, ps):
                nc.vector.tensor_copy(out=h5_sb[:, mo:mo + mw], in_=ps)

            spmm(xw5_full, K, [(0, K)], h5_drain)

            # -- heads --
            nc.vector.tensor_mul(out=h4sq[:], in0=h4aug[:], in1=h4aug[:])

            with tc.tile_pool(name="hps", bufs=1, space="PSUM") as hp:
                for mt in range(MT):
                    ms = slice(mt * 128, (mt + 1) * 128)
                    # q: d2 = |c|^2 - 2 h.c + |h|^2 ; q = norm(1/(1+d2))
                    psq = hp.tile([128, K], f32, tag="psq", name=f"psq_{mt}")
                    nc.tensor.matmul(psq[:], lhsT=h4aug[:, ms], rhs=qmat_sb[:],
                                     start=True, stop=False)
                    nc.tensor.matmul(psq[:], lhsT=h4sq[:, ms], rhs=ones_sb[:],
                                     start=False, stop=True)
                    tq = small.tile([128, K], f32, tag="tq", name=f"tq_{mt}")
                    nc.scalar.add(tq[:], psq[:], 1.0)
                    qn = small.tile([128, K], f32, tag="qn", name=f"qn_{mt}")
                    nc.vector.reciprocal(qn[:], tq[:])
                    s1 = small.tile([128, 1], f32, tag="s1", name=f"s1_{mt}")
                    nc.vector.reduce_sum(out=s1[:], in_=qn[:], axis=AX.X)
                    nc.vector.reciprocal(s1[:], s1[:])
                    qv = small.tile([128, K], f32, tag="qv", name=f"qv_{mt}")
                    nc.vector.tensor_scalar_mul(qv[:], qn[:], s1[:])
                    nc.sync.dma_start(q_o[ms, :], qv[:])

                    # predict = softmax(h5) over K
                    pst = hp.tile([128, K], f32, tag="pst", name=f"pst_{mt}")
                    nc.tensor.transpose(pst[:], h5_sb[:, ms], ident_sb[:])
                    mx = small.tile([128, 1], f32, tag="mx", name=f"mx_{mt}")
                    nc.vector.reduce_max(out=mx[:], in_=pst[:], axis=AX.X)
                    nc.vector.tensor_scalar_mul(mx[:], mx[:], -1.0)
                    ev = small.tile([128, K], f32, tag="ev", name=f"ev_{mt}")
                    nc.scalar.activation(ev[:], pst[:], AF.Exp, bias=mx[:])
                    s2 = small.tile([128, 1], f32, tag="s2", name=f"s2_{mt}")
                    nc.vector.reduce_sum(out=s2[:], in_=ev[:], axis=AX.X)
                    nc.vector.reciprocal(s2[:], s2[:])
                    pv = small.tile([128, K], f32, tag="pv", name=f"pv_{mt}")
                    nc.vector.tensor_scalar_mul(pv[:], ev[:], s2[:])
                    nc.sync.dma_start(pred_o[ms, :], pv[:])

    nc.compile()
    return nc


def _prep_inputs(inputs):
    """Shard + lay out full inputs for the 8 cores."""
    f32 = np.float32
    x = np.asarray(inputs["x"], f32)
    adj = np.asarray(inputs["adj"], f32)

    def b16(a):
        return np.ascontiguousarray(np.asarray(a, f32).astype(BF16))

    def col(a):
        return np.ascontiguousarray(np.asarray(a, f32).reshape(-1, 1))

    shared = {
        "e1w": b16(np.asarray(inputs["ae_enc1_w"], f32) * TS),
        "e2w": b16(inputs["ae_enc2_w"]),
        "e3w": b16(inputs["ae_enc3_w"]),
        "zw": b16(inputs["ae_z_w"]),
        "e1b": col(np.asarray(inputs["ae_enc1_b"], f32) * TS),
        "e2b": col(np.asarray(inputs["ae_enc2_b"], f32) * TS),
        "e3b": col(np.asarray(inputs["ae_enc3_b"], f32) * TS),
        "zb": col(np.asarray(inputs["ae_z_b"], f32) * TS),
        "g1w": b16(inputs["gnn1_w"]),
        "g2w": b16(np.asarray(inputs["gnn2_w"], f32) * (1.0 - SIGMA)),
        "g3w": b16(np.asarray(inputs["gnn3_w"], f32) * (1.0 - SIGMA)),
        "g4w": b16(np.asarray(inputs["gnn4_w"], f32) * (1.0 - SIGMA)),
    }
    g5 = np.zeros((128, K), f32)
    g5[:NZ] = np.asarray(inputs["gnn5_w"], f32) * (1.0 - SIGMA)
    shared["g5wp"] = b16(g5)
    fca = np.zeros((128, NI), f32)
    fca[:NZ] = np.asarray(inputs["fc1_w"], f32)
    fca[NZ] = np.asarray(inputs["fc1_b"], f32)
    shared["fc1aug"] = b16(fca)
    cl = np.asarray(inputs["cluster"], f32)  # [K, NZ]
    qm = np.zeros((128, K), f32)
    qm[:NZ] = -2.0 * cl.T
    qm[NZ] = (cl * cl).sum(axis=1)
    shared["qmat"] = np.ascontiguousarray(qm)

    in_maps = []
    for c in range(NCORES):
        r0 = c * R
        r1 = min(N, r0 + R)
        nreal = max(0, r1 - r0)
        xT = np.zeros((NI, R), BF16)
        if nreal > 0:
            xT[:, :nreal] = x[r0:r1].T.astype(BF16)
        adjT = np.zeros((NP, R), BF16)
        if nreal > 0:
            adjT[:N, :nreal] = adj[r0:r1].T.astype(BF16)
        m = dict(shared)
        m["xT"] = xT
        m["adjT"] = adjT
        in_maps.append(m)
    return in_maps


def kernel(**inputs):
    global _cached_nc
    from concourse.bass_utils import run_bass_kernel_spmd

    in_maps = _prep_inputs(inputs)
    if _cached_nc is None:
        _cached_nc = _build()
    res = run_bass_kernel_spmd(_cached_nc, in_maps, core_ids=list(range(NCORES)))
    outs = res.results
    x_bar = np.concatenate([o["xbar_o"] for o in outs], axis=0)[:N]
    q = np.concatenate([o["q_o"] for o in outs], axis=0)[:N]
    predict = np.concatenate([o["pred_o"] for o in outs], axis=0)[:N]
    z = np.concatenate([o["z_o"] for o in outs], axis=1).T[:N]
    z = np.ascontiguousarray(z)
    return (x_bar, q, predict, z)


# revision 5
# speedup vs baseline: 1.0620x; 1.0620x over previous
"""Trainium2 Bass kernel for nn_DGC (deep graph clustering GNN).

Full inputs in, full outputs out. Internally: row-shards the node dimension
N=10000 (padded to 10240) across 8 NeuronCores; adj is passed per-core as a
transposed (k-node major) block so the dense spmm adj @ (h @ W) maps directly
onto the PE array; per-layer activations (h @ W) are AllGathered across cores
in row chunks so comm pipelines with compute.

Dataflow (per core, zero on-device transposes):
  - activations kept feature-major [E, 1280]; AE chain = relu(W.T @ x.T)
  - GNN residual mix folded into host-scaled weights: stored tra' = (3/7)*tra,
    gnn_w' = 0.7*gnn_w, so u = h + tra' with a single DVE add
  - xw computed node-major via lhsT=u.T-tile, rhs=W -> chunked AllGather
  - spmm: lhsT = xw_full k-tile (stationary), rhs = adjT k-tile (moving),
    accumulated over 80 k-tiles into PSUM -> h.T feature-major
  - q head: d2 via augmented matmul [-2c.T; |c|^2] with ones row; predict via
    PE-transpose + free-dim softmax. Compute bf16, accumulate/heads f32.
"""

import numpy as np
import ml_dtypes

BF16 = ml_dtypes.bfloat16

NCORES = 8
N = 10000
NP = 10240
R = NP // NCORES          # 1280 rows per core
MT = R // 128             # 10 m-tiles
KT = NP // 128            # 80 k-tiles (spmm contraction)
NI, E1, E2, E3, NZ, K = 1024, 512, 512, 2048, 64, 16
SIGMA = 0.3
TS = SIGMA / (1.0 - SIGMA)   # tra storage scale (3/7)
MCH = [(0, 512), (512, 512), (1024, 256)]  # m-chunks of the 1280 local nodes
L3_SLABS = [(0, 768), (768, 768), (1536, 512)]
AGC = 5                   # AllGather row chunks (2 k-tiles per rank each)
CROWS = R // AGC          # 256 rows per rank per chunk

_cached_nc = None


def _build():
    import concourse.bacc as bacc
    import concourse.mybir as mybir
    import concourse.tile as tile
    from concourse.masks import make_identity
    from contextlib import ExitStack

    dt = mybir.dt
    AF = mybir.ActivationFunctionType
    AX = mybir.AxisListType

    nc = bacc.Bacc("TRN2", target_bir_lowering=False, debug=False,
                   num_devices=NCORES)

    f32, bf = dt.float32, dt.bfloat16
    xT_d = nc.dram_tensor("xT", [NI, R], bf, kind="ExternalInput")
    adjT_d = nc.dram_tensor("adjT", [NP, R], bf, kind="ExternalInput")
    e1w_d = nc.dram_tensor("e1w", [NI, E1], bf, kind="ExternalInput")
    e2w_d = nc.dram_tensor("e2w", [E1, E2], bf, kind="ExternalInput")
    e3w_d = nc.dram_tensor("e3w", [E2, E3], bf, kind="ExternalInput")
    zw_d = nc.dram_tensor("zw", [E3, NZ], bf, kind="ExternalInput")
    e1b_d = nc.dram_tensor("e1b", [E1, 1], f32, kind="ExternalInput")
    e2b_d = nc.dram_tensor("e2b", [E2, 1], f32, kind="ExternalInput")
    e3b_d = nc.dram_tensor("e3b", [E3, 1], f32, kind="ExternalInput")
    zb_d = nc.dram_tensor("zb", [NZ, 1], f32, kind="ExternalInput")
    g1w_d = nc.dram_tensor("g1w", [NI, E1], bf, kind="ExternalInput")
    g2w_d = nc.dram_tensor("g2w", [E1, E2], bf, kind="ExternalInput")
    g3w_d = nc.dram_tensor("g3w", [E2, E3], bf, kind="ExternalInput")
    g4w_d = nc.dram_tensor("g4w", [E3, NZ], bf, kind="ExternalInput")
    g5wp_d = nc.dram_tensor("g5wp", [128, K], bf, kind="ExternalInput")
    fc1aug_d = nc.dram_tensor("fc1aug", [128, NI], bf, kind="ExternalInput")
    qmat_d = nc.dram_tensor("qmat", [128, K], f32, kind="ExternalInput")

    xbar_o = nc.dram_tensor("xbar_o", [R, NI], f32, kind="ExternalOutput")
    q_o = nc.dram_tensor("q_o", [R, K], f32, kind="ExternalOutput")
    pred_o = nc.dram_tensor("pred_o", [R, K], f32, kind="ExternalOutput")
    z_o = nc.dram_tensor("z_o", [NZ, R], f32, kind="ExternalOutput")

    rg = [list(range(NCORES))]

    with tile.TileContext(nc) as tc:
        with ExitStack() as ctx:
            dram = ctx.enter_context(tc.tile_pool(name="dram", bufs=1, space="DRAM"))
            stage = ctx.enter_context(tc.tile_pool(name="stage", bufs=3))
            small = ctx.enter_context(tc.tile_pool(name="small", bufs=4))
            dpsum = ctx.enter_context(tc.tile_pool(name="dpsum", bufs=2, space="PSUM"))
            main = ctx.enter_context(tc.tile_pool(name="main", bufs=1))

            # ---- DRAM scratch ----
            def ag_bufs(name, e):
                loc = dram.tile([R, e], bf, name=f"{name}_loc")
                chunks = [dram.tile([NCORES * CROWS, e], bf, name=f"{name}_c{j}",
                                    addr_space="Shared") for j in range(AGC)]
                return loc, chunks

            xw1_loc, xw1_ch = ag_bufs("xw1", E1)
            xw2_loc, xw2_ch = ag_bufs("xw2", E2)
            xw3_loc, xw3_ch = ag_bufs("xw3", E3)
            xw4_loc = dram.tile([R, NZ], bf, name="xw4_loc")
            xw4_full = dram.tile([NP, NZ], bf, name="xw4_full", addr_space="Shared")
            xw5_loc = dram.tile([R, K], bf, name="xw5_loc")
            xw5_full = dram.tile([NP, K], bf, name="xw5_full", addr_space="Shared")
            tra3_dram = dram.tile([128, E3 // 128, R], bf, name="tra3_dram")
            u4_dram = dram.tile([128, E3 // 128, R], bf, name="u4_dram")

            # ---- long-lived small SBUF ----
            z_sb = main.tile([NZ, R], bf, name="z_sb")
            h4aug = main.tile([128, R], f32, name="h4aug")
            rh4_sb = main.tile([128, R], bf, name="rh4_sb")
            h5_sb = main.tile([16, R], f32, name="h5_sb")
            h4sq = main.tile([128, R], f32, name="h4sq")
            qmat_sb = main.tile([128, K], f32, name="qmat_sb")
            ones_sb = main.tile([128, K], f32, name="ones_sb")
            ident_sb = main.tile([16, 16], f32, name="ident_sb")

            nc.gpsimd.memset(h4aug[:], 0.0)
            nc.gpsimd.memset(h4aug[NZ:NZ + 1, :], 1.0)   # ones row for q matmul
            nc.gpsimd.memset(rh4_sb[:], 0.0)
            nc.gpsimd.memset(rh4_sb[NZ:NZ + 1, :], 1.0)  # ones row for fc1 bias
            nc.sync.dma_start(qmat_sb[:], qmat_d[:])
            nc.gpsimd.memset(ones_sb[:], 0.0)
            nc.gpsimd.memset(ones_sb[:NZ, :], 1.0)
            make_identity(nc, ident_sb[:])

            # ---- helpers ----
            def load_w(pool, w_d, kf_dim, eo):
                w = pool.tile([128, kf_dim // 128, eo], bf, name=f"w_{w_d.name}")
                nc.sync.dma_start(w[:], w_d.ap().rearrange("(o p) e -> p o e", p=128))
                return w

            def load_b(b_d, eo):
                nchunks = max(1, eo // 128)
                p = min(128, eo)
                b = small.tile([p, nchunks], f32, name=f"b_{b_d.name}", tag="bias")
                nc.sync.dma_start(b[:], b_d.ap().rearrange("(o p) x -> p (o x)", p=p))
                return b

            def dense_fm(in_sb, w_d, b_d, kf, eo, drain):
                """Feature-major dense: psum[n_chunk, m] = W.T @ in.T."""
                with tc.tile_pool(name=f"wp_{w_d.name}", bufs=1) as wp:
                    w_sb = load_w(wp, w_d, kf, eo)
                    b_sb = load_b(b_d, eo) if b_d is not None else None
                    for nci in range(eo // 128):
                        for (mo, mw) in MCH:
                            ps = dpsum.tile([128, 512], f32, tag="dps",
                                            name=f"dps_{w_d.name}_{nci}_{mo}")
                            for kfi in range(kf // 128):
                                nc.tensor.matmul(
                                    ps[:, :mw],
                                    lhsT=w_sb[:, kfi, nci * 128:(nci + 1) * 128],
                                    rhs=in_sb[:, kfi, mo:mo + mw],
                                    start=(kfi == 0), stop=(kfi == kf // 128 - 1))
                            bias_ap = b_sb[:, nci:nci + 1] if b_sb is not None else None
                            drain(nci, mo, mw, ps[:, :mw], bias_ap)

            def dense_nm(u_sb, w_sb, kf, eo, mt_drain, out_dt=bf):
                """Node-major dense: psum[m_tile, n] = u @ W; mt_drain per chunk."""
                for mt in range(MT):
                    st = stage.tile([128, max(eo, 128)], out_dt, tag="nmstage",
                                    name=f"nmst_{mt}")
                    for nco in range((eo + 511) // 512):
                        ncw = min(512, eo - nco * 512)
                        ps = dpsum.tile([128, 512], f32, tag="dps",
                                        name=f"nmps_{mt}_{nco}")
                        nkf = max(1, kf // 128)
                        for kfi in range(nkf):
                            lhs = u_sb[:, kfi, mt * 128:(mt + 1) * 128] if kf > 128 \
                                else u_sb[:, mt * 128:(mt + 1) * 128]
                            rhs = w_sb[:, kfi, nco * 512:nco * 512 + ncw] if kf > 128 \
                                else w_sb[:, nco * 512:nco * 512 + ncw]
                            nc.tensor.matmul(ps[:, :ncw], lhsT=lhs, rhs=rhs,
                                             start=(kfi == 0), stop=(kfi == nkf - 1))
                        mt_drain(mt, nco, ncw, ps[:, :ncw], st)

            def ag_chunked(loc, chunks, e):
                """Row-chunked AllGather; returns src(ko, soff, scols) for spmm."""
                for j in range(AGC):
                    nc.gpsimd.collective_compute(
                        "AllGather", mybir.AluOpType.bypass, replica_groups=rg,
                        ins=[loc[j * CROWS:(j + 1) * CROWS, :].opt()],
                        outs=[chunks[j].opt()])

                def src(ko, soff, scols):
                    r, i = divmod(ko, MT)
                    j, t = divmod(i, 2)
                    row = r * CROWS + t * 128
                    return chunks[j][row:row + 128, soff:soff + scols]
                return src

            def ag_single(loc, full):
                nc.gpsimd.collective_compute(
                    "AllGather", mybir.AluOpType.bypass, replica_groups=rg,
                    ins=[loc.opt()], outs=[full.opt()])

                def src(ko, soff, scols):
                    return full[ko * 128:(ko + 1) * 128, soff:soff + scols]
                return src

            adjT_r = adjT_d.ap().rearrange("(o p) m -> p o m", p=128)

            def spmm(src, e, slabs, drain, adj_bufs=3):
                """h.T[n, m] = sum_k xw_full[k, n] * adjT[k, m] (column-sliced)."""
                with (
                    tc.tile_pool(name=f"slab_{e}", bufs=1) as slp,
                    tc.tile_pool(name=f"adjp_{e}", bufs=adj_bufs) as adp,
                    tc.tile_pool(name=f"psp_{e}", bufs=1, space="PSUM") as pp,
                ):
                    for (soff, scols) in slabs:
                        slab = slp.tile([128, KT, scols], bf, tag="slab",
                                        name=f"slab_{e}_{soff}")
                        for ko in range(KT):
                            nc.sync.dma_start(slab[:, ko, :], src(ko, soff, scols))
                        G = max(1, scols // 128)
                        gw = min(128, scols)
                        for (mo, mw) in MCH:
                            pss = [pp.tile([128, 512], f32, tag=f"ps{g}",
                                           name=f"ps_{e}_{soff}_{mo}_{g}")[:gw, :mw]
                                   for g in range(G)]
                            for ko in range(0, KT, 2):
                                adj2 = adp.tile([128, 2, 512], bf, tag="adj",
                                                name=f"adj_{e}_{soff}_{mo}_{ko}")
                                nc.sync.dma_start(
                                    adj2[:, :, :mw],
                                    adjT_r[:, ko:ko + 2, mo:mo + mw])
                                for kk in range(2):
                                    k = ko + kk
                                    for g in range(G):
                                        nc.tensor.matmul(
                                            pss[g],
                                            lhsT=slab[:, k, g * gw:(g + 1) * gw],
                                            rhs=adj2[:, kk, :mw],
                                            start=(k == 0), stop=(k == KT - 1))
                            for g in range(G):
                                drain(soff // 128 + g, mo, mw, pss[g])

            def spmm_fw(src, e, drain, adj_bufs=6):
                """Full-width spmm for small e (G=1): one adjT row read per
                k-pair, all three m-chunks accumulate concurrently."""
                with (
                    tc.tile_pool(name=f"slab_{e}", bufs=1) as slp,
                    tc.tile_pool(name=f"adjp_{e}", bufs=adj_bufs) as adp,
                    tc.tile_pool(name=f"psp_{e}", bufs=1, space="PSUM") as pp,
                ):
                    slab = slp.tile([128, KT, e], bf, tag="slab", name=f"slab_{e}")
                    for ko in range(KT):
                        nc.sync.dma_start(slab[:, ko, :], src(ko, 0, e))
                    pss = [pp.tile([128, 512], f32, tag=f"ps{ci}",
                                   name=f"ps_{e}_{ci}")[:e, :mw]
                           for ci, (mo, mw) in enumerate(MCH)]
                    for ko in range(0, KT, 2):
                        adjr = adp.tile([128, 2, R], bf, tag="adj",
                                        name=f"adjr_{e}_{ko}")
                        nc.sync.dma_start(adjr[:], adjT_r[:, ko:ko + 2, :])
                        for kk in range(2):
                            k = ko + kk
                            for ci, (mo, mw) in enumerate(MCH):
                                nc.tensor.matmul(
                                    pss[ci], lhsT=slab[:, k, :e],
                                    rhs=adjr[:, kk, mo:mo + mw],
                                    start=(k == 0), stop=(k == KT - 1))
                    for ci, (mo, mw) in enumerate(MCH):
                        drain(0, mo, mw, pss[ci])

            # ================= Program =================

            with ExitStack() as phaseA:
                pA = phaseA.enter_context(tc.tile_pool(name="pA", bufs=1))
                tra1_sb = pA.tile([128, E1 // 128, R], bf, name="tra1_sb")
                tra2_sb = pA.tile([128, E2 // 128, R], bf, name="tra2_sb")
                h1_sb = pA.tile([128, E1 // 128, R], bf, name="h1_sb")
                h2_sb = pA.tile([128, E2 // 128, R], bf, name="h2_sb")

                # -- xw1 = x @ g1w (node-major) -> chunked AG; AE enc1 --
                with tc.tile_pool(name="pX", bufs=1) as pX:
                    xT_sb = pX.tile([128, NI // 128, R], bf, name="xT_sb")
                    nc.sync.dma_start(
                        xT_sb[:], xT_d.ap().rearrange("(o p) m -> p o m", p=128))

                    with tc.tile_pool(name="wg1", bufs=1) as wp:
                        g1w_sb = load_w(wp, g1w_d, NI, E1)

                        def xw1_drain(mt, nco, ncw, ps, st):
                            nc.scalar.copy(st[:, nco * 512:nco * 512 + ncw], ps)
                            if nco == (E1 + 511) // 512 - 1:
                                nc.sync.dma_start(
                                    xw1_loc[mt * 128:(mt + 1) * 128, :], st[:, :E1])

                        dense_nm(xT_sb, g1w_sb, NI, E1, xw1_drain)
                    src1 = ag_chunked(xw1_loc, xw1_ch, E1)

                    def fm_relu_drain(out_sb):
                        def d(nci, mo, mw, ps, b):
                            nc.scalar.activation(out_sb[:, nci, mo:mo + mw], ps,
                                                 AF.Relu,
                                                 bias=b if b is not None else 0.0)
                        return d

                    dense_fm(xT_sb, e1w_d, e1b_d, NI, E1, fm_relu_drain(tra1_sb))

                dense_fm(tra1_sb, e2w_d, e2b_d, E1, E2, fm_relu_drain(tra2_sb))

                def tra3_drain(nci, mo, mw, ps, b):
                    st = stage.tile([128, 512], bf, tag="fmstage",
                                    name=f"t3st_{nci}_{mo}")
                    nc.scalar.activation(st[:, :mw], ps, AF.Relu,
                                         bias=b if b is not None else 0.0)
                    nc.sync.dma_start(tra3_dram[:, nci, mo:mo + mw], st[:, :mw])

                dense_fm(tra2_sb, e3w_d, e3b_d, E2, E3, tra3_drain)

                # -- z' = tra3' @ zw + zb' (stream tra3' back from DRAM) --
                with tc.tile_pool(name="wz", bufs=1) as wp, \
                     tc.tile_pool(name="t3s", bufs=3) as t3p:
                    zw_sb = load_w(wp, zw_d, E3, NZ)
                    zb_sb = load_b(zb_d, NZ)
                    for (mo, mw) in MCH:
                        ps = dpsum.tile([128, 512], f32, tag="dps", name=f"zps_{mo}")
                        for kfi in range(E3 // 128):
                            t3t = t3p.tile([128, 512], bf, tag="t3t",
                                           name=f"t3t_{mo}_{kfi}")
                            nc.sync.dma_start(t3t[:, :mw],
                                              tra3_dram[:, kfi, mo:mo + mw])
                            nc.tensor.matmul(ps[:NZ, :mw], lhsT=zw_sb[:, kfi, :NZ],
                                             rhs=t3t[:, :mw], start=(kfi == 0),
                                             stop=(kfi == E3 // 128 - 1))
                        nc.scalar.activation(z_sb[:, mo:mo + mw], ps[:NZ, :mw],
                                             AF.Identity, bias=zb_sb[:, 0:1])
                        zst = stage.tile([NZ, 512], f32, tag="zstage",
                                         name=f"zst_{mo}")
                        nc.scalar.activation(zst[:, :mw], ps[:NZ, :mw], AF.Identity,
                                             bias=zb_sb[:, 0:1])
                        nc.scalar.mul(zst[:, :mw], zst[:, :mw], 1.0 / TS)
                        nc.sync.dma_start(z_o[:, mo:mo + mw], zst[:, :mw])

                # -- spmm1: h1 = relu(adj @ xw1) --
                def h_drain(out_sb):
                    def d(gg, mo, mw, ps):
                        nc.scalar.activation(out_sb[:, gg, mo:mo + mw], ps, AF.Relu)
                    return d

                spmm(src1, E1, [(0, E1)], h_drain(h1_sb))

                # -- u2 = h1 + tra1'; xw2 -> AG --
                for kfi in range(E1 // 128):
                    nc.vector.tensor_add(out=h1_sb[:, kfi, :],
                                         in0=h1_sb[:, kfi, :],
                                         in1=tra1_sb[:, kfi, :])
                with tc.tile_pool(name="wg2", bufs=1) as wp:
                    g2w_sb = load_w(wp, g2w_d, E1, E2)

                    def xw2_drain(mt, nco, ncw, ps, st):
                        nc.scalar.copy(st[:, nco * 512:nco * 512 + ncw], ps)
                        if nco == (E2 + 511) // 512 - 1:
                            nc.sync.dma_start(
                                xw2_loc[mt * 128:(mt + 1) * 128, :], st[:, :E2])

                    dense_nm(h1_sb, g2w_sb, E1, E2, xw2_drain)
                src2 = ag_chunked(xw2_loc, xw2_ch, E2)

                # -- spmm2: h2 --
                spmm(src2, E2, [(0, E2)], h_drain(h2_sb))

                # -- u3 = h2 + tra2'; xw3 -> AG --
                for kfi in range(E2 // 128):
                    nc.vector.tensor_add(out=h2_sb[:, kfi, :],
                                         in0=h2_sb[:, kfi, :],
                                         in1=tra2_sb[:, kfi, :])
                with tc.tile_pool(name="wg3", bufs=1) as wp:
                    g3w_sb = load_w(wp, g3w_d, E2, E3)

                    def xw3_drain(mt, nco, ncw, ps, st):
                        nc.scalar.copy(st[:, nco * 512:nco * 512 + ncw], ps)
                        if nco == (E3 + 511) // 512 - 1:
                            nc.sync.dma_start(
                                xw3_loc[mt * 128:(mt + 1) * 128, :], st[:, :E3])

                    dense_nm(h2_sb, g3w_sb, E2, E3, xw3_drain)
                src3 = ag_chunked(xw3_loc, xw3_ch, E3)

            # -- spmm3 with fused u4 = relu(h3) + tra3' -> u4_dram --
            with tc.tile_pool(name="t3r", bufs=4) as t3r:
                def h3u4_drain(gg, mo, mw, ps):
                    st = stage.tile([128, 512], bf, tag="fmstage",
                                    name=f"h3st_{gg}_{mo}")
                    nc.scalar.activation(st[:, :mw], ps, AF.Relu)
                    t3 = t3r.tile([128, 512], bf, tag="t3rt",
                                  name=f"t3r_{gg}_{mo}")
                    nc.sync.dma_start(t3[:, :mw], tra3_dram[:, gg, mo:mo + mw])
                    nc.vector.tensor_add(out=st[:, :mw], in0=st[:, :mw],
                                         in1=t3[:, :mw])
                    nc.sync.dma_start(u4_dram[:, gg, mo:mo + mw], st[:, :mw])

                spmm(src3, E3, L3_SLABS, h3u4_drain)

            # -- xw4 = u4 @ g4w -> AG --
            with ExitStack() as phaseB:
                pB = phaseB.enter_context(tc.tile_pool(name="pB", bufs=1))
                u4_sb = pB.tile([128, E3 // 128, R], bf, name="u4_sb")
                for kfi in range(E3 // 128):
                    nc.sync.dma_start(u4_sb[:, kfi, :], u4_dram[:, kfi, :])
                with tc.tile_pool(name="wg4", bufs=1) as wp:
                    g4w_sb = load_w(wp, g4w_d, E3, NZ)

                    def xw4_drain(mt, nco, ncw, ps, st):
                        nc.scalar.copy(st[:, :ncw], ps)
                        nc.sync.dma_start(xw4_loc[mt * 128:(mt + 1) * 128, :],
                                          st[:, :NZ])

                    dense_nm(u4_sb, g4w_sb, E3, NZ, xw4_drain)
                src4 = ag_single(xw4_loc, xw4_full)

            # -- spmm4: h4 (no relu) -> h4aug f32 + relu(h4) bf16 --
            def h4_drain(gg, mo, mw, ps):
                nc.vector.tensor_copy(out=h4aug[:NZ, mo:mo + mw], in_=ps)
                nc.scalar.activation(rh4_sb[:NZ, mo:mo + mw], ps, AF.Relu)

            spmm_fw(src4, NZ, h4_drain)

            # -- u5 = relu(h4) + z'; xw5 -> AG (critical path first) --
            with ExitStack() as phaseC:
                pC = phaseC.enter_context(tc.tile_pool(name="pC", bufs=1))
                u5_sb = pC.tile([128, R], bf, name="u5_sb")
                nc.gpsimd.memset(u5_sb[:], 0.0)
                nc.vector.tensor_add(out=u5_sb[:NZ, :], in0=rh4_sb[:NZ, :],
                                     in1=z_sb[:])
                with tc.tile_pool(name="wg5", bufs=1) as wp:
                    g5w_sb = wp.tile([128, K], bf, name="g5w_sb")
                    nc.sync.dma_start(g5w_sb[:], g5wp_d[:])

                    def xw5_drain(mt, nco, ncw, ps, st):
                        nc.scalar.copy(st[:, :ncw], ps)
                        nc.sync.dma_start(xw5_loc[mt * 128:(mt + 1) * 128, :],
                                          st[:, :K])

                    dense_nm(u5_sb, g5w_sb, 128, K, xw5_drain)
                src5 = ag_single(xw5_loc, xw5_full)

            # -- x_bar = relu(relu(h4) @ fc1_w + b) (fills AG5 gap) --
            with tc.tile_pool(name="wfc", bufs=1) as wp:
                fc1_sb = wp.tile([128, NI], bf, name="fc1_sb")
                nc.sync.dma_start(fc1_sb[:], fc1aug_d[:])

                def fc1_drain(mt, nco, ncw, ps, st):
                    nc.scalar.activation(st[:, nco * 512:nco * 512 + ncw], ps,
                                         AF.Relu)
                    if nco == (NI + 511) // 512 - 1:
                        nc.sync.dma_start(xbar_o[mt * 128:(mt + 1) * 128, :],
                                          st[:, :NI])

                dense_nm(rh4_sb, fc1_sb, 128, NI, fc1_drain, out_dt=f32)

            # -- q head (depends only on h4; also fills AG5 gap) --
            nc.vector.tensor_mul(out=h4sq[:], in0=h4aug[:], in1=h4aug[:])
            with tc.tile_pool(name="hq", bufs=2, space="PSUM") as hq:
                for mt in range(MT):
                    ms = slice(mt * 128, (mt + 1) * 128)
                    psq = hq.tile([128, K], f32, tag="psq", name=f"psq_{mt}")
                    nc.tensor.matmul(psq[:], lhsT=h4aug[:, ms], rhs=qmat_sb[:],
                                     start=True, stop=False)
                    nc.tensor.matmul(psq[:], lhsT=h4sq[:, ms], rhs=ones_sb[:],
                                     start=False, stop=True)
                    tq = small.tile([128, K], f32, tag="tq", name=f"tq_{mt}")
                    nc.scalar.add(tq[:], psq[:], 1.0)
                    qn = small.tile([128, K], f32, tag="qn", name=f"qn_{mt}")
                    nc.vector.reciprocal(qn[:], tq[:])
                    s1 = small.tile([128, 1], f32, tag="s1", name=f"s1_{mt}")
                    nc.vector.reduce_sum(out=s1[:], in_=qn[:], axis=AX.X)
                    nc.vector.reciprocal(s1[:], s1[:])
                    qv = small.tile([128, K], f32, tag="qv", name=f"qv_{mt}")
                    nc.vector.tensor_scalar_mul(qv[:], qn[:], s1[:])
                    nc.sync.dma_start(q_o[ms, :], qv[:])

            # -- spmm5 with fused predict softmax per m-chunk --
            with tc.tile_pool(name="hp", bufs=2, space="PSUM") as hp:
                def h5_drain(gg, mo, mw, ps):
                    nc.vector.tensor_copy(out=h5_sb[:, mo:mo + mw], in_=ps)
                    for mt in range(mo // 128, (mo + mw) // 128):
                        ms = slice(mt * 128, (mt + 1) * 128)
                        pst = hp.tile([128, K], f32, tag="pst", name=f"pst_{mt}")
                        nc.tensor.transpose(pst[:], h5_sb[:, ms], ident_sb[:])
                        mx = small.tile([128, 1], f32, tag="mx", name=f"mx_{mt}")
                        nc.vector.reduce_max(out=mx[:], in_=pst[:], axis=AX.X)
                        nc.vector.tensor_scalar_mul(mx[:], mx[:], -1.0)
                        ev = small.tile([128, K], f32, tag="ev", name=f"ev_{mt}")
                        nc.scalar.activation(ev[:], pst[:], AF.Exp, bias=mx[:])
                        s2 = small.tile([128, 1], f32, tag="s2", name=f"s2_{mt}")
                        nc.vector.reduce_sum(out=s2[:], in_=ev[:], axis=AX.X)
                        nc.vector.reciprocal(s2[:], s2[:])
                        pv = small.tile([128, K], f32, tag="pv", name=f"pv_{mt}")
                        nc.vector.tensor_scalar_mul(pv[:], ev[:], s2[:])
                        nc.sync.dma_start(pred_o[ms, :], pv[:])

                spmm_fw(src5, K, h5_drain)

    nc.compile()
    return nc


def _prep_inputs(inputs):
    """Shard + lay out full inputs for the 8 cores."""
    f32 = np.float32
    x = np.asarray(inputs["x"], f32)
    adj = np.asarray(inputs["adj"], f32)

    def b16(a):
        return np.ascontiguousarray(np.asarray(a, f32).astype(BF16))

    def col(a):
        return np.ascontiguousarray(np.asarray(a, f32).reshape(-1, 1))

    shared = {
        "e1w": b16(np.asarray(inputs["ae_enc1_w"], f32) * TS),
        "e2w": b16(inputs["ae_enc2_w"]),
        "e3w": b16(inputs["ae_enc3_w"]),
        "zw": b16(inputs["ae_z_w"]),
        "e1b": col(np.asarray(inputs["ae_enc1_b"], f32) * TS),
        "e2b": col(np.asarray(inputs["ae_enc2_b"], f32) * TS),
        "e3b": col(np.asarray(inputs["ae_enc3_b"], f32) * TS),
        "zb": col(np.asarray(inputs["ae_z_b"], f32) * TS),
        "g1w": b16(inputs["gnn1_w"]),
        "g2w": b16(np.asarray(inputs["gnn2_w"], f32) * (1.0 - SIGMA)),
        "g3w": b16(np.asarray(inputs["gnn3_w"], f32) * (1.0 - SIGMA)),
        "g4w": b16(np.asarray(inputs["gnn4_w"], f32) * (1.0 - SIGMA)),
    }
    g5 = np.zeros((128, K), f32)
    g5[:NZ] = np.asarray(inputs["gnn5_w"], f32) * (1.0 - SIGMA)
    shared["g5wp"] = b16(g5)
    fca = np.zeros((128, NI), f32)
    fca[:NZ] = np.asarray(inputs["fc1_w"], f32)
    fca[NZ] = np.asarray(inputs["fc1_b"], f32)
    shared["fc1aug"] = b16(fca)
    cl = np.asarray(inputs["cluster"], f32)  # [K, NZ]
    qm = np.zeros((128, K), f32)
    qm[:NZ] = -2.0 * cl.T
    qm[NZ] = (cl * cl).sum(axis=1)
    shared["qmat"] = np.ascontiguousarray(qm)

    in_maps = []
    for c in range(NCORES):
        r0 = c * R
        r1 = min(N, r0 + R)
        nreal = max(0, r1 - r0)
        xT = np.zeros((NI, R), BF16)
        if nreal > 0:
            xT[:, :nreal] = x[r0:r1].T.astype(BF16)
        adjT = np.zeros((NP, R), BF16)
        if nreal > 0:
            adjT[:N, :nreal] = adj[r0:r1].T.astype(BF16)
        m = dict(shared)
        m["xT"] = xT
        m["adjT"] = adjT
        in_maps.append(m)
    return in_maps


def kernel(**inputs):
    global _cached_nc
    from concourse.bass_utils import run_bass_kernel_spmd

    in_maps = _prep_inputs(inputs)
    if _cached_nc is None:
        _cached_nc = _build()
    res = run_bass_kernel_spmd(_cached_nc, in_maps, core_ids=list(range(NCORES)))
    outs = res.results
    x_bar = np.concatenate([o["xbar_o"] for o in outs], axis=0)[:N]
    q = np.concatenate([o["q_o"] for o in outs], axis=0)[:N]
    predict = np.concatenate([o["pred_o"] for o in outs], axis=0)[:N]
    z = np.concatenate([o["z_o"] for o in outs], axis=1).T[:N]
    z = np.ascontiguousarray(z)
    return (x_bar, q, predict, z)


# revision 10
# speedup vs baseline: 1.1173x; 1.0521x over previous
"""Trainium2 Bass kernel for nn_DGC (deep graph clustering GNN).

Full inputs in, full outputs out. Row-shards the node dimension N=10000
(padded to 10240) across 8 NeuronCores; adj is passed per-core as a
transposed (k-node major) block so the dense spmm adj @ (h @ W) maps directly
onto the PE array; per-layer activations (h @ W) are AllGathered across cores
in three row chunks, launched from inside the previous spmm so comm hides
under compute.

Per core, zero on-device transposes:
  - activations feature-major [E, 1280]; AE chain = relu(W.T @ x.T)
  - GNN residual mix folded into host-scaled weights: stored tra' = (3/7)*tra,
    gnn_w' = 0.7*gnn_w, so u = h + tra' is one DVE add
  - xw computed node-major via lhsT=u.T-tile, rhs=W -> chunked AllGather
  - spmm: lhsT = xw_full k-tile (stationary), rhs = adjT k-tile (moving),
    80 k-tiles accumulated in PSUM -> h.T feature-major. k consumption is
    ordered by AG chunk so each spmm starts as soon as chunk 0 lands.
  - q head: d2 via augmented matmul [-2c.T; |c|^2] with ones row; predict via
    PE-transpose + free-dim softmax. Compute bf16, accumulate/heads f32.
DMA queues: adjT stream on nc.sync; everything else on nc.scalar.
"""

import numpy as np
import ml_dtypes

BF16 = ml_dtypes.bfloat16

NCORES = 8
N = 10000
NP = 10240
R = NP // NCORES          # 1280 rows per core
MT = R // 128             # 10 m-tiles
KT = NP // 128            # 80 k-tiles (spmm contraction)
NI, E1, E2, E3, NZ, K = 1024, 512, 512, 2048, 64, 16
SIGMA = 0.3
TS = SIGMA / (1.0 - SIGMA)   # tra storage scale (3/7)
MCH = [(0, 512), (512, 512), (1024, 256)]  # m-chunks of the 1280 local nodes
# AG row chunks per rank (aligned with MCH): i-tile ranges and widths
CW = [512, 512, 256]
IOFF = [0, 4, 8]
# spmm k consumption order: AG-chunk phase -> i-pair -> rank
K_SEQ = [r * MT + ia for ia in (0, 2, 4, 6, 8) for r in range(NCORES)]

_cached_nc = None


def _build():
    import concourse.bacc as bacc
    import concourse.mybir as mybir
    import concourse.tile as tile
    from concourse.masks import make_identity
    from contextlib import ExitStack

    dt = mybir.dt
    AF = mybir.ActivationFunctionType
    AX = mybir.AxisListType

    nc = bacc.Bacc("TRN2", target_bir_lowering=False, debug=False,
                   num_devices=NCORES)

    f32, bf = dt.float32, dt.bfloat16
    xT_d = nc.dram_tensor("xT", [NI, R], bf, kind="ExternalInput")
    adjT_d = nc.dram_tensor("adjT", [NP, R], bf, kind="ExternalInput")
    e1w_d = nc.dram_tensor("e1w", [NI, E1], bf, kind="ExternalInput")
    e2w_d = nc.dram_tensor("e2w", [E1, E2], bf, kind="ExternalInput")
    e3w_d = nc.dram_tensor("e3w", [E2, E3], bf, kind="ExternalInput")
    zw_d = nc.dram_tensor("zw", [E3, NZ], bf, kind="ExternalInput")
    e1b_d = nc.dram_tensor("e1b", [E1, 1], f32, kind="ExternalInput")
    e2b_d = nc.dram_tensor("e2b", [E2, 1], f32, kind="ExternalInput")
    e3b_d = nc.dram_tensor("e3b", [E3, 1], f32, kind="ExternalInput")
    zb_d = nc.dram_tensor("zb", [NZ, 1], f32, kind="ExternalInput")
    g1w_d = nc.dram_tensor("g1w", [NI, E1], bf, kind="ExternalInput")
    g2w_d = nc.dram_tensor("g2w", [E1, E2], bf, kind="ExternalInput")
    g3w_d = nc.dram_tensor("g3w", [E2, E3], bf, kind="ExternalInput")
    g4w_d = nc.dram_tensor("g4w", [E3, NZ], bf, kind="ExternalInput")
    g5wp_d = nc.dram_tensor("g5wp", [128, K], bf, kind="ExternalInput")
    fc1aug_d = nc.dram_tensor("fc1aug", [128, NI], bf, kind="ExternalInput")
    qmat_d = nc.dram_tensor("qmat", [128, K], f32, kind="ExternalInput")

    xbar_o = nc.dram_tensor("xbar_o", [R, NI], f32, kind="ExternalOutput")
    q_o = nc.dram_tensor("q_o", [R, K], f32, kind="ExternalOutput")
    pred_o = nc.dram_tensor("pred_o", [R, K], f32, kind="ExternalOutput")
    z_o = nc.dram_tensor("z_o", [NZ, R], f32, kind="ExternalOutput")

    rg = [list(range(NCORES))]

    with tile.TileContext(nc) as tc:
        with ExitStack() as ctx:
            dram = ctx.enter_context(tc.tile_pool(name="dram", bufs=1, space="DRAM"))
            stage = ctx.enter_context(tc.tile_pool(name="stage", bufs=3))
            small = ctx.enter_context(tc.tile_pool(name="small", bufs=4))
            dpsum = ctx.enter_context(tc.tile_pool(name="dpsum", bufs=2, space="PSUM"))
            psp = ctx.enter_context(tc.tile_pool(name="psp", bufs=1, space="PSUM"))
            slabp = ctx.enter_context(tc.tile_pool(name="slabp", bufs=1))
            adjp = ctx.enter_context(tc.tile_pool(name="adjp", bufs=3))
            main = ctx.enter_context(tc.tile_pool(name="main", bufs=1))

            # ---- DRAM scratch ----
            def ag_bufs(name, e):
                loc = dram.tile([R, e], bf, name=f"{name}_loc")
                chunks = [dram.tile([NCORES * CW[c], e], bf, name=f"{name}_c{c}",
                                    addr_space="Shared") for c in range(3)]
                return loc, chunks

            xw1_loc, xw1_ch = ag_bufs("xw1", E1)
            xw2_loc, xw2_ch = ag_bufs("xw2", E2)
            xw3_loc, xw3_ch = ag_bufs("xw3", E3)
            xw4_loc = dram.tile([R, NZ], bf, name="xw4_loc")
            xw4_full = dram.tile([NP, NZ], bf, name="xw4_full", addr_space="Shared")
            xw5_loc = dram.tile([R, K], bf, name="xw5_loc")
            xw5_full = dram.tile([NP, K], bf, name="xw5_full", addr_space="Shared")
            tra3_dram = dram.tile([128, E3 // 128, R], bf, name="tra3_dram")
            u4_dram = dram.tile([128, E3 // 128, R], bf, name="u4_dram")

            # ---- long-lived small SBUF ----
            z_sb = main.tile([NZ, R], bf, name="z_sb")
            h4aug = main.tile([128, R], f32, name="h4aug")
            rh4_sb = main.tile([128, R], bf, name="rh4_sb")
            h5_sb = main.tile([16, R], f32, name="h5_sb")
            h4sq = main.tile([128, R], f32, name="h4sq")
            qmat_sb = main.tile([128, K], f32, name="qmat_sb")
            ones_sb = main.tile([128, K], f32, name="ones_sb")
            ident_sb = main.tile([16, 16], f32, name="ident_sb")

            nc.gpsimd.memset(h4aug[:], 0.0)
            nc.gpsimd.memset(h4aug[NZ:NZ + 1, :], 1.0)   # ones row for q matmul
            nc.gpsimd.memset(rh4_sb[:], 0.0)
            nc.gpsimd.memset(rh4_sb[NZ:NZ + 1, :], 1.0)  # ones row for fc1 bias
            nc.scalar.dma_start(qmat_sb[:], qmat_d[:])
            nc.gpsimd.memset(ones_sb[:], 0.0)
            nc.gpsimd.memset(ones_sb[:NZ, :], 1.0)
            make_identity(nc, ident_sb[:])

            # ---- helpers ----
            def load_w(pool, w_d, kf_dim, eo):
                w = pool.tile([128, kf_dim // 128, eo], bf, name=f"w_{w_d.name}")
                nc.scalar.dma_start(w[:], w_d.ap().rearrange("(o p) e -> p o e", p=128))
                return w

            def load_b(b_d, eo):
                nchunks = max(1, eo // 128)
                p = min(128, eo)
                b = small.tile([p, nchunks], f32, name=f"b_{b_d.name}", tag="bias")
                nc.scalar.dma_start(b[:], b_d.ap().rearrange("(o p) x -> p (o x)", p=p))
                return b

            def dense_fm(in_sb, w_d, b_d, kf, eo, drain):
                """Feature-major dense: psum[n_chunk, m] = W.T @ in.T."""
                with tc.tile_pool(name=f"wp_{w_d.name}", bufs=1) as wp:
                    w_sb = load_w(wp, w_d, kf, eo)
                    b_sb = load_b(b_d, eo) if b_d is not None else None
                    for nci in range(eo // 128):
                        for (mo, mw) in MCH:
                            ps = dpsum.tile([128, 512], f32, tag="dps",
                                            name=f"dps_{w_d.name}_{nci}_{mo}")
                            for kfi in range(kf // 128):
                                nc.tensor.matmul(
                                    ps[:, :mw],
                                    lhsT=w_sb[:, kfi, nci * 128:(nci + 1) * 128],
                                    rhs=in_sb[:, kfi, mo:mo + mw],
                                    start=(kfi == 0), stop=(kfi == kf // 128 - 1))
                            bias_ap = b_sb[:, nci:nci + 1] if b_sb is not None else None
                            drain(nci, mo, mw, ps[:, :mw], bias_ap)

            def dense_nm(u_sb, w_sb, kf, eo, mt_drain, out_dt=bf, mts=None):
                """Node-major dense: psum[m_tile, n] = u @ W."""
                for mt in (range(MT) if mts is None else mts):
                    for nco in range((eo + 511) // 512):
                        ncw = min(512, eo - nco * 512)
                        ps = dpsum.tile([128, 512], f32, tag="dps",
                                        name=f"nmps_{mt}_{nco}")
                        nkf = max(1, kf // 128)
                        for kfi in range(nkf):
                            lhs = u_sb[:, kfi, mt * 128:(mt + 1) * 128] if kf > 128 \
                                else u_sb[:, mt * 128:(mt + 1) * 128]
                            rhs = w_sb[:, kfi, nco * 512:nco * 512 + ncw] if kf > 128 \
                                else w_sb[:, nco * 512:nco * 512 + ncw]
                            nc.tensor.matmul(ps[:, :ncw], lhsT=lhs, rhs=rhs,
                                             start=(kfi == 0), stop=(kfi == nkf - 1))
                        st = stage.tile([128, 512], out_dt, tag="nmstage",
                                        name=f"nmst_{mt}_{nco}")
                        mt_drain(mt, nco, ncw, ps[:, :ncw], st)

            def ag_chunk(loc, chunks, c):
                r0 = (0, 512, 1024)[c]
                nc.gpsimd.collective_compute(
                    "AllGather", mybir.AluOpType.bypass, replica_groups=rg,
                    ins=[loc[r0:r0 + CW[c], :].opt()], outs=[chunks[c].opt()])

            def ag_single(loc, full):
                nc.gpsimd.collective_compute(
                    "AllGather", mybir.AluOpType.bypass, replica_groups=rg,
                    ins=[loc.opt()], outs=[full.opt()])

            def src_chunked(chunks):
                def src(ko, soff, scols):
                    r, i = divmod(ko, MT)
                    c = 0 if i < 4 else (1 if i < 8 else 2)
                    row = r * CW[c] + 128 * (i - IOFF[c])
                    return chunks[c][row:row + 256, soff:soff + scols] \
                        .rearrange("(b p) e -> p b e", p=128)
                return src

            def src_full(full):
                def src(ko, soff, scols):
                    return full[ko * 128:(ko + 2) * 128, soff:soff + scols] \
                        .rearrange("(b p) e -> p b e", p=128)
                return src

            adjT_r = adjT_d.ap().rearrange("(o p) m -> p o m", p=128)

            def spmm(src, e, passes, drain, mc_done=None):
                """h.T[n, m] = sum_k xw_full[k, n] * adjT[k, m] (column passes)."""
                last_p = len(K_SEQ) - 1
                for pi, (soff, scols) in enumerate(passes):
                    qts = [slabp.tile([128, 20, 512], bf, tag=f"slabq{q}",
                                      name=f"sq_{e}_{soff}_{q}") for q in range(4)]
                    for p, ko in enumerate(K_SEQ):
                        q, j = divmod(p, 10)
                        nc.scalar.dma_start(qts[q][:, 2 * j:2 * j + 2, :scols],
                                            src(ko, soff, scols))
                    G = max(1, scols // 128)
                    gw = min(128, scols)
                    for ci, (mo, mw) in enumerate(MCH):
                        pss = [psp.tile([128, 512], f32, tag=f"ps{g}",
                                        name=f"ps_{e}_{soff}_{mo}_{g}")[:gw, :mw]
                               for g in range(G)]
                        for p, ko in enumerate(K_SEQ):
                            q, j = divmod(p, 10)
                            adj2 = adjp.tile([128, 2, 1280], bf, tag="adj",
                                             name=f"adj_{e}_{soff}_{mo}_{ko}")
                            nc.sync.dma_start(adj2[:, :, :mw],
                                              adjT_r[:, ko:ko + 2, mo:mo + mw])
                            for kk in range(2):
                                for g in range(G):
                                    nc.tensor.matmul(
                                        pss[g],
                                        lhsT=qts[q][:, 2 * j + kk,
                                                    g * gw:(g + 1) * gw],
                                        rhs=adj2[:, kk, :mw],
                                        start=(p == 0 and kk == 0),
                                        stop=(p == last_p and kk == 1))
                        for g in range(G):
                            drain(soff // 128 + g, mo, mw, pss[g])
                        if mc_done is not None and pi == len(passes) - 1:
                            mc_done(ci, mo, mw)

            def spmm_fw(src, e, drain):
                """Full-width spmm for small e (G=1): one adjT row read per
                k-pair, all three m-chunks accumulate concurrently."""
                qts = [slabp.tile([128, 20, e], bf, tag=f"slabq{q}",
                                  name=f"sqf_{e}_{q}") for q in range(4)]
                for p, ko in enumerate(K_SEQ):
                    q, j = divmod(p, 10)
                    nc.scalar.dma_start(qts[q][:, 2 * j:2 * j + 2, :], src(ko, 0, e))
                pss = [psp.tile([128, 512], f32, tag=f"ps{ci}",
                                name=f"psf_{e}_{ci}")[:e, :mw]
                       for ci, (mo, mw) in enumerate(MCH)]
                last_p = len(K_SEQ) - 1
                for p, ko in enumerate(K_SEQ):
                    q, j = divmod(p, 10)
                    adjr = adjp.tile([128, 2, 1280], bf, tag="adj",
                                     name=f"adjr_{e}_{ko}")
                    nc.sync.dma_start(adjr[:], adjT_r[:, ko:ko + 2, :])
                    for kk in range(2):
                        for ci, (mo, mw) in enumerate(MCH):
                            nc.tensor.matmul(
                                pss[ci], lhsT=qts[q][:, 2 * j + kk, :e],
                                rhs=adjr[:, kk, mo:mo + mw],
                                start=(p == 0 and kk == 0),
                                stop=(p == last_p and kk == 1))
                for ci, (mo, mw) in enumerate(MCH):
                    drain(0, mo, mw, pss[ci])

            # ================= Program =================

            xT_r = xT_d.ap().rearrange("(o p) m -> p o m", p=128)

            with ExitStack() as phaseA:
                pA = phaseA.enter_context(tc.tile_pool(name="pA", bufs=1))
                tra1_sb = pA.tile([128, E1 // 128, R], bf, name="tra1_sb")
                tra2_sb = pA.tile([128, E2 // 128, R], bf, name="tra2_sb")
                h1_sb = pA.tile([128, E1 // 128, R], bf, name="h1_sb")
                h2_sb = pA.tile([128, E2 // 128, R], bf, name="h2_sb")

                # -- xw1 = x @ g1w (node-major, xT streamed); AG chunks per mc --
                with tc.tile_pool(name="wg1", bufs=1) as wp, \
                     tc.tile_pool(name="xts", bufs=3) as xp:
                    g1w_sb = load_w(wp, g1w_d, NI, E1)
                    for mt in range(MT):
                        st = stage.tile([128, E1], bf, tag="nmstage",
                                        name=f"x1st_{mt}")
                        ps = dpsum.tile([128, 512], f32, tag="dps",
                                        name=f"x1ps_{mt}")
                        for kfi in range(NI // 128):
                            xt = xp.tile([128, 128], bf, tag="xt1",
                                         name=f"xt1_{mt}_{kfi}")
                            nc.scalar.dma_start(
                                xt[:], xT_r[:, kfi, mt * 128:(mt + 1) * 128])
                            nc.tensor.matmul(ps[:], lhsT=xt[:],
                                             rhs=g1w_sb[:, kfi, :],
                                             start=(kfi == 0),
                                             stop=(kfi == NI // 128 - 1))
                        nc.scalar.copy(st[:], ps[:])
                        nc.scalar.dma_start(xw1_loc[mt * 128:(mt + 1) * 128, :],
                                            st[:])
                        if mt in (3, 7, 9):
                            ag_chunk(xw1_loc, xw1_ch, {3: 0, 7: 1, 9: 2}[mt])

                # -- AE enc1 (xT streamed, nci-inner with 4 psum banks) --
                with tc.tile_pool(name="we1", bufs=1) as wp, \
                     tc.tile_pool(name="xts2", bufs=3) as xp:
                    e1w_sb = load_w(wp, e1w_d, NI, E1)
                    e1b_sb = load_b(e1b_d, E1)
                    for (mo, mw) in MCH:
                        pss = [psp.tile([128, 512], f32, tag=f"ps{n}",
                                        name=f"e1ps_{mo}_{n}")[:, :mw]
                               for n in range(4)]
                        for kfi in range(NI // 128):
                            xt = xp.tile([128, 512], bf, tag="xt2",
                                         name=f"xt2_{mo}_{kfi}")
                            nc.scalar.dma_start(xt[:, :mw],
                                                xT_r[:, kfi, mo:mo + mw])
                            for n in range(4):
                                nc.tensor.matmul(
                                    pss[n], lhsT=e1w_sb[:, kfi,
                                                        n * 128:(n + 1) * 128],
                                    rhs=xt[:, :mw], start=(kfi == 0),
                                    stop=(kfi == NI // 128 - 1))
                        for n in range(4):
                            nc.scalar.activation(tra1_sb[:, n, mo:mo + mw],
                                                 pss[n], AF.Relu,
                                                 bias=e1b_sb[:, n:n + 1])

                def fm_relu_drain(out_sb):
                    def d(nci, mo, mw, ps, b):
                        nc.scalar.activation(out_sb[:, nci, mo:mo + mw], ps,
                                             AF.Relu,
                                             bias=b if b is not None else 0.0)
                    return d

                dense_fm(tra1_sb, e2w_d, e2b_d, E1, E2, fm_relu_drain(tra2_sb))

                def tra3_drain(nci, mo, mw, ps, b):
                    st = stage.tile([128, 512], bf, tag="fmstage",
                                    name=f"t3st_{nci}_{mo}")
                    nc.scalar.activation(st[:, :mw], ps, AF.Relu,
                                         bias=b if b is not None else 0.0)
                    nc.scalar.dma_start(tra3_dram[:, nci, mo:mo + mw], st[:, :mw])

                dense_fm(tra2_sb, e3w_d, e3b_d, E2, E3, tra3_drain)

                # -- z' = tra3' @ zw + zb' (stream tra3' back) --
                with tc.tile_pool(name="wz", bufs=1) as wp, \
                     tc.tile_pool(name="t3s", bufs=3) as t3p:
                    zw_sb = load_w(wp, zw_d, E3, NZ)
                    zb_sb = load_b(zb_d, NZ)
                    for (mo, mw) in MCH:
                        ps = dpsum.tile([128, 512], f32, tag="dps", name=f"zps_{mo}")
                        for kfi in range(E3 // 128):
                            t3t = t3p.tile([128, 512], bf, tag="t3t",
                                           name=f"t3t_{mo}_{kfi}")
                            nc.scalar.dma_start(t3t[:, :mw],
                                                tra3_dram[:, kfi, mo:mo + mw])
                            nc.tensor.matmul(ps[:NZ, :mw], lhsT=zw_sb[:, kfi, :NZ],
                                             rhs=t3t[:, :mw], start=(kfi == 0),
                                             stop=(kfi == E3 // 128 - 1))
                        nc.scalar.activation(z_sb[:, mo:mo + mw], ps[:NZ, :mw],
                                             AF.Identity, bias=zb_sb[:, 0:1])
                        zst = stage.tile([NZ, 512], f32, tag="zstage",
                                         name=f"zst_{mo}")
                        nc.scalar.activation(zst[:, :mw], ps[:NZ, :mw], AF.Identity,
                                             bias=zb_sb[:, 0:1])
                        nc.scalar.mul(zst[:, :mw], zst[:, :mw], 1.0 / TS)
                        nc.scalar.dma_start(z_o[:, mo:mo + mw], zst[:, :mw])

                def h_drain(out_sb):
                    def d(gg, mo, mw, ps):
                        nc.scalar.activation(out_sb[:, gg, mo:mo + mw], ps, AF.Relu)
                    return d

                # -- spmm1 + fused u2/xw2/AG2 per m-chunk --
                with tc.tile_pool(name="wg2", bufs=1) as wp:
                    g2w_sb = load_w(wp, g2w_d, E1, E2)

                    def xw2_drain(mt, nco, ncw, ps, st):
                        nc.scalar.copy(st[:, :ncw], ps)
                        nc.scalar.dma_start(
                            xw2_loc[mt * 128:(mt + 1) * 128,
                                    nco * 512:nco * 512 + ncw], st[:, :ncw])

                    def mc1_done(ci, mo, mw):
                        for kfi in range(E1 // 128):
                            nc.vector.tensor_add(
                                out=h1_sb[:, kfi, mo:mo + mw],
                                in0=h1_sb[:, kfi, mo:mo + mw],
                                in1=tra1_sb[:, kfi, mo:mo + mw])
                        dense_nm(h1_sb, g2w_sb, E1, E2, xw2_drain,
                                 mts=range(mo // 128, (mo + mw) // 128))
                        ag_chunk(xw2_loc, xw2_ch, ci)

                    spmm(src_chunked(xw1_ch), E1, [(0, E1)], h_drain(h1_sb),
                         mc_done=mc1_done)

                # -- spmm2 + fused u3/xw3/AG3 per m-chunk --
                with tc.tile_pool(name="wg3", bufs=1) as wp:
                    g3w_sb = load_w(wp, g3w_d, E2, E3)

                    def xw3_drain(mt, nco, ncw, ps, st):
                        nc.scalar.copy(st[:, :ncw], ps)
                        nc.scalar.dma_start(
                            xw3_loc[mt * 128:(mt + 1) * 128,
                                    nco * 512:nco * 512 + ncw], st[:, :ncw])

                    def mc2_done(ci, mo, mw):
                        for kfi in range(E2 // 128):
                            nc.vector.tensor_add(
                                out=h2_sb[:, kfi, mo:mo + mw],
                                in0=h2_sb[:, kfi, mo:mo + mw],
                                in1=tra2_sb[:, kfi, mo:mo + mw])
                        dense_nm(h2_sb, g3w_sb, E2, E3, xw3_drain,
                                 mts=range(mo // 128, (mo + mw) // 128))
                        ag_chunk(xw3_loc, xw3_ch, ci)

                    spmm(src_chunked(xw2_ch), E2, [(0, E2)], h_drain(h2_sb),
                         mc_done=mc2_done)

            # -- spmm3 with fused u4 = relu(h3) + tra3' -> u4_dram --
            with tc.tile_pool(name="t3r", bufs=4) as t3r:
                def h3u4_drain(gg, mo, mw, ps):
                    st = stage.tile([128, 512], bf, tag="fmstage",
                                    name=f"h3st_{gg}_{mo}")
                    nc.scalar.activation(st[:, :mw], ps, AF.Relu)
                    t3 = t3r.tile([128, 512], bf, tag="t3rt",
                                  name=f"t3r_{gg}_{mo}")
                    nc.scalar.dma_start(t3[:, :mw], tra3_dram[:, gg, mo:mo + mw])
                    nc.vector.tensor_add(out=st[:, :mw], in0=st[:, :mw],
                                         in1=t3[:, :mw])
                    nc.scalar.dma_start(u4_dram[:, gg, mo:mo + mw], st[:, :mw])

                spmm(src_chunked(xw3_ch), E3,
                     [(0, 512), (512, 512), (1024, 512), (1536, 512)], h3u4_drain)

            # -- xw4 = u4 @ g4w -> AG --
            with ExitStack() as phaseB:
                pB = phaseB.enter_context(tc.tile_pool(name="pB", bufs=1))
                u4_sb = pB.tile([128, E3 // 128, R], bf, name="u4_sb")
                for kfi in range(E3 // 128):
                    nc.scalar.dma_start(u4_sb[:, kfi, :], u4_dram[:, kfi, :])
                with tc.tile_pool(name="wg4", bufs=1) as wp:
                    g4w_sb = load_w(wp, g4w_d, E3, NZ)

                    def xw4_drain(mt, nco, ncw, ps, st):
                        nc.scalar.copy(st[:, :ncw], ps)
                        nc.scalar.dma_start(xw4_loc[mt * 128:(mt + 1) * 128, :],
                                            st[:, :NZ])

                    dense_nm(u4_sb, g4w_sb, E3, NZ, xw4_drain)
                ag_single(xw4_loc, xw4_full)

            # -- spmm4: h4 (no relu) -> h4aug f32 + relu(h4) bf16 --
            def h4_drain(gg, mo, mw, ps):
                nc.vector.tensor_copy(out=h4aug[:NZ, mo:mo + mw], in_=ps)
                nc.scalar.activation(rh4_sb[:NZ, mo:mo + mw], ps, AF.Relu)

            spmm_fw(src_full(xw4_full), NZ, h4_drain)

            # -- u5 = relu(h4) + z'; xw5 -> AG (critical path first) --
            with ExitStack() as phaseC:
                pC = phaseC.enter_context(tc.tile_pool(name="pC", bufs=1))
                u5_sb = pC.tile([128, R], bf, name="u5_sb")
                nc.gpsimd.memset(u5_sb[:], 0.0)
                nc.vector.tensor_add(out=u5_sb[:NZ, :], in0=rh4_sb[:NZ, :],
                                     in1=z_sb[:])
                with tc.tile_pool(name="wg5", bufs=1) as wp:
                    g5w_sb = wp.tile([128, K], bf, name="g5w_sb")
                    nc.scalar.dma_start(g5w_sb[:], g5wp_d[:])

                    def xw5_drain(mt, nco, ncw, ps, st):
                        nc.scalar.copy(st[:, :ncw], ps)
                        nc.scalar.dma_start(xw5_loc[mt * 128:(mt + 1) * 128, :],
                                            st[:, :K])

                    dense_nm(u5_sb, g5w_sb, 128, K, xw5_drain)
                ag_single(xw5_loc, xw5_full)

            # -- x_bar = relu(relu(h4) @ fc1_w + b) (fills AG5 gap) --
            with tc.tile_pool(name="wfc", bufs=1) as wp:
                fc1_sb = wp.tile([128, NI], bf, name="fc1_sb")
                nc.scalar.dma_start(fc1_sb[:], fc1aug_d[:])

                def fc1_drain(mt, nco, ncw, ps, st):
                    nc.scalar.activation(st[:, :ncw], ps, AF.Relu)
                    nc.scalar.dma_start(
                        xbar_o[mt * 128:(mt + 1) * 128,
                               nco * 512:nco * 512 + ncw], st[:, :ncw])

                dense_nm(rh4_sb, fc1_sb, 128, NI, fc1_drain, out_dt=f32)

            # -- q head (depends only on h4; also fills AG5 gap) --
            nc.vector.tensor_mul(out=h4sq[:], in0=h4aug[:], in1=h4aug[:])
            with tc.tile_pool(name="hq", bufs=2, space="PSUM") as hq:
                for mt in range(MT):
                    ms = slice(mt * 128, (mt + 1) * 128)
                    psq = hq.tile([128, K], f32, tag="psq", name=f"psq_{mt}")
                    nc.tensor.matmul(psq[:], lhsT=h4aug[:, ms], rhs=qmat_sb[:],
                                     start=True, stop=False)
                    nc.tensor.matmul(psq[:], lhsT=h4sq[:, ms], rhs=ones_sb[:],
                                     start=False, stop=True)
                    tq = small.tile([128, K], f32, tag="tq", name=f"tq_{mt}")
                    nc.scalar.add(tq[:], psq[:], 1.0)
                    qn = small.tile([128, K], f32, tag="qn", name=f"qn_{mt}")
                    nc.vector.reciprocal(qn[:], tq[:])
                    s1 = small.tile([128, 1], f32, tag="s1", name=f"s1_{mt}")
                    nc.vector.reduce_sum(out=s1[:], in_=qn[:], axis=AX.X)
                    nc.vector.reciprocal(s1[:], s1[:])
                    qv = small.tile([128, K], f32, tag="qv", name=f"qv_{mt}")
                    nc.vector.tensor_scalar_mul(qv[:], qn[:], s1[:])
                    nc.scalar.dma_start(q_o[ms, :], qv[:])

            # -- spmm5 with fused predict softmax per m-chunk --
            with tc.tile_pool(name="hp", bufs=2, space="PSUM") as hp:
                def h5_drain(gg, mo, mw, ps):
                    nc.vector.tensor_copy(out=h5_sb[:, mo:mo + mw], in_=ps)
                    for mt in range(mo // 128, (mo + mw) // 128):
                        ms = slice(mt * 128, (mt + 1) * 128)
                        pst = hp.tile([128, K], f32, tag="pst", name=f"pst_{mt}")
                        nc.tensor.transpose(pst[:], h5_sb[:, ms], ident_sb[:])
                        mx = small.tile([128, 1], f32, tag="mx", name=f"mx_{mt}")
                        nc.vector.reduce_max(out=mx[:], in_=pst[:], axis=AX.X)
                        nc.vector.tensor_scalar_mul(mx[:], mx[:], -1.0)
                        ev = small.tile([128, K], f32, tag="ev", name=f"ev_{mt}")
                        nc.scalar.activation(ev[:], pst[:], AF.Exp, bias=mx[:])
                        s2 = small.tile([128, 1], f32, tag="s2", name=f"s2_{mt}")
                        nc.vector.reduce_sum(out=s2[:], in_=ev[:], axis=AX.X)
                        nc.vector.reciprocal(s2[:], s2[:])
                        pv = small.tile([128, K], f32, tag="pv", name=f"pv_{mt}")
                        nc.vector.tensor_scalar_mul(pv[:], ev[:], s2[:])
                        nc.scalar.dma_start(pred_o[ms, :], pv[:])

                spmm_fw(src_full(xw5_full), K, h5_drain)

    nc.compile()
    return nc


def _prep_inputs(inputs):
    """Shard + lay out full inputs for the 8 cores."""
    f32 = np.float32
    x = np.asarray(inputs["x"], f32)
    adj = np.asarray(inputs["adj"], f32)

    def b16(a):
        return np.ascontiguousarray(np.asarray(a, f32).astype(BF16))

    def col(a):
        return np.ascontiguousarray(np.asarray(a, f32).reshape(-1, 1))

    shared = {
        "e1w": b16(np.asarray(inputs["ae_enc1_w"], f32) * TS),
        "e2w": b16(inputs["ae_enc2_w"]),
        "e3w": b16(inputs["ae_enc3_w"]),
        "zw": b16(inputs["ae_z_w"]),
        "e1b": col(np.asarray(inputs["ae_enc1_b"], f32) * TS),
        "e2b": col(np.asarray(inputs["ae_enc2_b"], f32) * TS),
        "e3b": col(np.asarray(inputs["ae_enc3_b"], f32) * TS),
        "zb": col(np.asarray(inputs["ae_z_b"], f32) * TS),
        "g1w": b16(inputs["gnn1_w"]),
        "g2w": b16(np.asarray(inputs["gnn2_w"], f32) * (1.0 - SIGMA)),
        "g3w": b16(np.asarray(inputs["gnn3_w"], f32) * (1.0 - SIGMA)),
        "g4w": b16(np.asarray(inputs["gnn4_w"], f32) * (1.0 - SIGMA)),
    }
    g5 = np.zeros((128, K), f32)
    g5[:NZ] = np.asarray(inputs["gnn5_w"], f32) * (1.0 - SIGMA)
    shared["g5wp"] = b16(g5)
    fca = np.zeros((128, NI), f32)
    fca[:NZ] = np.asarray(inputs["fc1_w"], f32)
    fca[NZ] = np.asarray(inputs["fc1_b"], f32)
    shared["fc1aug"] = b16(fca)
    cl = np.asarray(inputs["cluster"], f32)  # [K, NZ]
    qm = np.zeros((128, K), f32)
    qm[:NZ] = -2.0 * cl.T
    qm[NZ] = (cl * cl).sum(axis=1)
    shared["qmat"] = np.ascontiguousarray(qm)

    in_maps = []
    for c in range(NCORES):
        r0 = c * R
        r1 = min(N, r0 + R)
        nreal = max(0, r1 - r0)
        xT = np.zeros((NI, R), BF16)
        if nreal > 0:
            xT[:, :nreal] = x[r0:r1].T.astype(BF16)
        adjT = np.zeros((NP, R), BF16)
        if nreal > 0:
            adjT[:N, :nreal] = adj[r0:r1].T.astype(BF16)
        m = dict(shared)
        m["xT"] = xT
        m["adjT"] = adjT
        in_maps.append(m)
    return in_maps


def kernel(**inputs):
    global _cached_nc
    from concourse.bass_utils import run_bass_kernel_spmd

    in_maps = _prep_inputs(inputs)
    if _cached_nc is None:
        _cached_nc = _build()
    res = run_bass_kernel_spmd(_cached_nc, in_maps, core_ids=list(range(NCORES)))
    outs = res.results
    x_bar = np.concatenate([o["xbar_o"] for o in outs], axis=0)[:N]
    q = np.concatenate([o["q_o"] for o in outs], axis=0)[:N]
    predict = np.concatenate([o["pred_o"] for o in outs], axis=0)[:N]
    z = np.concatenate([o["z_o"] for o in outs], axis=1).T[:N]
    z = np.ascontiguousarray(z)
    return (x_bar, q, predict, z)


# revision 14
# speedup vs baseline: 1.1482x; 1.0277x over previous
"""Trainium2 Bass kernel for nn_DGC (deep graph clustering GNN).

Full inputs in, full outputs out. Row-shards the node dimension N=10000
(padded to 10240) across 8 NeuronCores; adj is passed per-core as a
transposed (k-node major) block so the dense spmm adj @ (h @ W) maps directly
onto the PE array; per-layer activations (h @ W) are AllGathered across cores
in three row chunks, launched from inside the previous spmm so comm hides
under compute.

Per core, zero on-device transposes:
  - activations feature-major [E, 1280]; AE chain = relu(W.T @ x.T)
  - GNN residual mix folded into host-scaled weights: stored tra' = (3/7)*tra,
    gnn_w' = 0.7*gnn_w, so u = h + tra' is one DVE add
  - xw computed node-major via lhsT=u.T-tile, rhs=W -> chunked AllGather
  - spmm: lhsT = xw_full k-tile (stationary), rhs = adjT k-tile (moving),
    80 k-tiles accumulated in PSUM -> h.T feature-major. k consumption is
    ordered by AG chunk so each spmm starts as soon as chunk 0 lands.
  - q head: d2 via augmented matmul [-2c.T; |c|^2] with ones row; predict via
    PE-transpose + free-dim softmax. Compute bf16, accumulate/heads f32.
DMA queues: adjT stream on nc.sync; everything else on nc.scalar.
"""

import numpy as np
import ml_dtypes

BF16 = ml_dtypes.bfloat16

NCORES = 8
N = 10000
NP = 10240
R = NP // NCORES          # 1280 rows per core
MT = R // 128             # 10 m-tiles
KT = NP // 128            # 80 k-tiles (spmm contraction)
NI, E1, E2, E3, NZ, K = 1024, 512, 512, 2048, 64, 16
SIGMA = 0.3
TS = SIGMA / (1.0 - SIGMA)   # tra storage scale (3/7)
MCH = [(0, 512), (512, 512), (1024, 256)]  # m-chunks of the 1280 local nodes
# AG row chunks per rank (aligned with MCH): i-tile ranges and widths
CW = [512, 512, 256]
IOFF = [0, 4, 8]
# spmm k consumption order: AG-chunk phase -> i-pair -> rank
K_SEQ = [r * MT + ia for ia in (0, 2, 4, 6, 8) for r in range(NCORES)]

_cached_nc = None


def _build():
    import concourse.bacc as bacc
    import concourse.mybir as mybir
    import concourse.tile as tile
    from concourse.masks import make_identity
    from contextlib import ExitStack

    dt = mybir.dt
    AF = mybir.ActivationFunctionType
    AX = mybir.AxisListType

    nc = bacc.Bacc("TRN2", target_bir_lowering=False, debug=False,
                   num_devices=NCORES)

    f32, bf = dt.float32, dt.bfloat16
    xT_d = nc.dram_tensor("xT", [NI, R], bf, kind="ExternalInput")
    adjT_d = nc.dram_tensor("adjT", [NP, R], bf, kind="ExternalInput")
    e1w_d = nc.dram_tensor("e1w", [NI, E1], bf, kind="ExternalInput")
    e2w_d = nc.dram_tensor("e2w", [E1, E2], bf, kind="ExternalInput")
    e3w_d = nc.dram_tensor("e3w", [E2, E3], bf, kind="ExternalInput")
    zw_d = nc.dram_tensor("zw", [E3, NZ], bf, kind="ExternalInput")
    e1b_d = nc.dram_tensor("e1b", [E1, 1], f32, kind="ExternalInput")
    e2b_d = nc.dram_tensor("e2b", [E2, 1], f32, kind="ExternalInput")
    e3b_d = nc.dram_tensor("e3b", [E3, 1], f32, kind="ExternalInput")
    zb_d = nc.dram_tensor("zb", [NZ, 1], f32, kind="ExternalInput")
    g1w_d = nc.dram_tensor("g1w", [NI, E1], bf, kind="ExternalInput")
    g2w_d = nc.dram_tensor("g2w", [E1, E2], bf, kind="ExternalInput")
    g3w_d = nc.dram_tensor("g3w", [E2, E3], bf, kind="ExternalInput")
    g4w_d = nc.dram_tensor("g4w", [E3, NZ], bf, kind="ExternalInput")
    g5wp_d = nc.dram_tensor("g5wp", [128, K], bf, kind="ExternalInput")
    fc1aug_d = nc.dram_tensor("fc1aug", [128, NI], bf, kind="ExternalInput")
    qmat_d = nc.dram_tensor("qmat", [128, K], f32, kind="ExternalInput")

    xbar_o = nc.dram_tensor("xbar_o", [R, NI], f32, kind="ExternalOutput")
    q_o = nc.dram_tensor("q_o", [R, K], f32, kind="ExternalOutput")
    pred_o = nc.dram_tensor("pred_o", [R, K], f32, kind="ExternalOutput")
    z_o = nc.dram_tensor("z_o", [NZ, R], f32, kind="ExternalOutput")

    rg = [list(range(NCORES))]

    with tile.TileContext(nc) as tc:
        with ExitStack() as ctx:
            dram = ctx.enter_context(tc.tile_pool(name="dram", bufs=1, space="DRAM"))
            stage = ctx.enter_context(tc.tile_pool(name="stage", bufs=3))
            small = ctx.enter_context(tc.tile_pool(name="small", bufs=4))
            dpsum = ctx.enter_context(tc.tile_pool(name="dpsum", bufs=2, space="PSUM"))
            psp = ctx.enter_context(tc.tile_pool(name="psp", bufs=1, space="PSUM"))
            slabp = ctx.enter_context(tc.tile_pool(name="slabp", bufs=1))
            adjp = ctx.enter_context(tc.tile_pool(name="adjp", bufs=3))
            main = ctx.enter_context(tc.tile_pool(name="main", bufs=1))

            # ---- DRAM scratch ----
            def ag_bufs(name, e):
                loc = dram.tile([R, e], bf, name=f"{name}_loc")
                chunks = [dram.tile([NCORES * CW[c], e], bf, name=f"{name}_c{c}",
                                    addr_space="Shared") for c in range(3)]
                return loc, chunks

            xw1_loc, xw1_ch = ag_bufs("xw1", E1)
            xw2_loc, xw2_ch = ag_bufs("xw2", E2)
            xw3_loc, xw3_ch = ag_bufs("xw3", E3)
            xw4_loc = dram.tile([R, NZ], bf, name="xw4_loc")
            xw4_full = dram.tile([NP, NZ], bf, name="xw4_full", addr_space="Shared")
            xw5_loc = dram.tile([R, K], bf, name="xw5_loc")
            xw5_full = dram.tile([NP, K], bf, name="xw5_full", addr_space="Shared")
            tra3_dram = dram.tile([128, E3 // 128, R], bf, name="tra3_dram")
            u4_dram = dram.tile([128, E3 // 128, R], bf, name="u4_dram")

            # ---- long-lived small SBUF ----
            z_sb = main.tile([NZ, R], bf, name="z_sb")
            h4aug = main.tile([128, R], f32, name="h4aug")
            rh4_sb = main.tile([128, R], bf, name="rh4_sb")
            h5_sb = main.tile([16, R], f32, name="h5_sb")
            h4sq = main.tile([128, R], f32, name="h4sq")
            qmat_sb = main.tile([128, K], f32, name="qmat_sb")
            ones_sb = main.tile([128, K], f32, name="ones_sb")
            ident_sb = main.tile([16, 16], f32, name="ident_sb")

            nc.gpsimd.memset(h4aug[:], 0.0)
            nc.gpsimd.memset(h4aug[NZ:NZ + 1, :], 1.0)   # ones row for q matmul
            nc.gpsimd.memset(rh4_sb[:], 0.0)
            nc.gpsimd.memset(rh4_sb[NZ:NZ + 1, :], 1.0)  # ones row for fc1 bias
            nc.scalar.dma_start(qmat_sb[:], qmat_d[:])
            nc.gpsimd.memset(ones_sb[:], 0.0)
            nc.gpsimd.memset(ones_sb[:NZ, :], 1.0)
            make_identity(nc, ident_sb[:])

            # ---- helpers ----
            def load_w(pool, w_d, kf_dim, eo):
                w = pool.tile([128, kf_dim // 128, eo], bf, name=f"w_{w_d.name}")
                nc.scalar.dma_start(w[:], w_d.ap().rearrange("(o p) e -> p o e", p=128))
                return w

            def load_b(b_d, eo):
                nchunks = max(1, eo // 128)
                p = min(128, eo)
                b = small.tile([p, nchunks], f32, name=f"b_{b_d.name}", tag="bias")
                nc.scalar.dma_start(b[:], b_d.ap().rearrange("(o p) x -> p (o x)", p=p))
                return b

            def dense_fm(in_sb, w_d, b_d, kf, eo, drain):
                """Feature-major dense: psum[n_chunk, m] = W.T @ in.T."""
                with tc.tile_pool(name=f"wp_{w_d.name}", bufs=1) as wp:
                    w_sb = load_w(wp, w_d, kf, eo)
                    b_sb = load_b(b_d, eo) if b_d is not None else None
                    for nci in range(eo // 128):
                        for (mo, mw) in MCH:
                            ps = dpsum.tile([128, 512], f32, tag="dps",
                                            name=f"dps_{w_d.name}_{nci}_{mo}")
                            for kfi in range(kf // 128):
                                nc.tensor.matmul(
                                    ps[:, :mw],
                                    lhsT=w_sb[:, kfi, nci * 128:(nci + 1) * 128],
                                    rhs=in_sb[:, kfi, mo:mo + mw],
                                    start=(kfi == 0), stop=(kfi == kf // 128 - 1))
                            bias_ap = b_sb[:, nci:nci + 1] if b_sb is not None else None
                            drain(nci, mo, mw, ps[:, :mw], bias_ap)

            def dense_nm(u_sb, w_sb, kf, eo, mt_drain, out_dt=bf, mts=None):
                """Node-major dense: psum[m_tile, n] = u @ W."""
                for mt in (range(MT) if mts is None else mts):
                    for nco in range((eo + 511) // 512):
                        ncw = min(512, eo - nco * 512)
                        ps = dpsum.tile([128, 512], f32, tag="dps",
                                        name=f"nmps_{mt}_{nco}")
                        nkf = max(1, kf // 128)
                        for kfi in range(nkf):
                            lhs = u_sb[:, kfi, mt * 128:(mt + 1) * 128] if kf > 128 \
                                else u_sb[:, mt * 128:(mt + 1) * 128]
                            rhs = w_sb[:, kfi, nco * 512:nco * 512 + ncw] if kf > 128 \
                                else w_sb[:, nco * 512:nco * 512 + ncw]
                            nc.tensor.matmul(ps[:, :ncw], lhsT=lhs, rhs=rhs,
                                             start=(kfi == 0), stop=(kfi == nkf - 1))
                        st = stage.tile([128, 512], out_dt, tag="nmstage",
                                        name=f"nmst_{mt}_{nco}")
                        mt_drain(mt, nco, ncw, ps[:, :ncw], st)

            def ag_chunk(loc, chunks, c):
                r0 = (0, 512, 1024)[c]
                nc.gpsimd.collective_compute(
                    "AllGather", mybir.AluOpType.bypass, replica_groups=rg,
                    ins=[loc[r0:r0 + CW[c], :].opt()], outs=[chunks[c].opt()])

            def ag_single(loc, full):
                nc.gpsimd.collective_compute(
                    "AllGather", mybir.AluOpType.bypass, replica_groups=rg,
                    ins=[loc.opt()], outs=[full.opt()])

            def src_chunked(chunks):
                def src(ko, soff, scols):
                    r, i = divmod(ko, MT)
                    c = 0 if i < 4 else (1 if i < 8 else 2)
                    row = r * CW[c] + 128 * (i - IOFF[c])
                    return chunks[c][row:row + 256, soff:soff + scols] \
                        .rearrange("(b p) e -> p b e", p=128)
                return src

            def src_full(full):
                def src(ko, soff, scols):
                    return full[ko * 128:(ko + 2) * 128, soff:soff + scols] \
                        .rearrange("(b p) e -> p b e", p=128)
                return src

            adjT_r = adjT_d.ap().rearrange("(o p) m -> p o m", p=128)

            def spmm(src, e, passes, drain, mc_done=None):
                """h.T[n, m] = sum_k xw_full[k, n] * adjT[k, m] (column passes)."""
                last_p = len(K_SEQ) - 1
                for pi, (soff, scols) in enumerate(passes):
                    qts = [slabp.tile([128, 20, 512], bf, tag=f"slabq{q}",
                                      name=f"sq_{e}_{soff}_{q}") for q in range(4)]
                    for p, ko in enumerate(K_SEQ):
                        q, j = divmod(p, 10)
                        nc.scalar.dma_start(qts[q][:, 2 * j:2 * j + 2, :scols],
                                            src(ko, soff, scols))
                    G = max(1, scols // 128)
                    gw = min(128, scols)
                    for ci, (mo, mw) in enumerate(MCH):
                        pss = [psp.tile([128, 512], f32, tag=f"ps{g}",
                                        name=f"ps_{e}_{soff}_{mo}_{g}")[:gw, :mw]
                               for g in range(G)]
                        for p, ko in enumerate(K_SEQ):
                            q, j = divmod(p, 10)
                            adj2 = adjp.tile([128, 2, 1280], bf, tag="adj",
                                             name=f"adj_{e}_{soff}_{mo}_{ko}")
                            nc.sync.dma_start(adj2[:, :, :mw],
                                              adjT_r[:, ko:ko + 2, mo:mo + mw])
                            for kk in range(2):
                                for g in range(G):
                                    nc.tensor.matmul(
                                        pss[g],
                                        lhsT=qts[q][:, 2 * j + kk,
                                                    g * gw:(g + 1) * gw],
                                        rhs=adj2[:, kk, :mw],
                                        start=(p == 0 and kk == 0),
                                        stop=(p == last_p and kk == 1))
                        for g in range(G):
                            drain(soff // 128 + g, mo, mw, pss[g])
                        if mc_done is not None and pi == len(passes) - 1:
                            mc_done(ci, mo, mw)

            def spmm_fw(src, e, drain):
                """Full-width spmm for small e (G=1): one adjT row read per
                k-pair, all three m-chunks accumulate concurrently."""
                qts = [slabp.tile([128, 20, e], bf, tag=f"slabq{q}",
                                  name=f"sqf_{e}_{q}") for q in range(4)]
                for p, ko in enumerate(K_SEQ):
                    q, j = divmod(p, 10)
                    nc.scalar.dma_start(qts[q][:, 2 * j:2 * j + 2, :], src(ko, 0, e))
                pss = [psp.tile([128, 512], f32, tag=f"ps{ci}",
                                name=f"psf_{e}_{ci}")[:e, :mw]
                       for ci, (mo, mw) in enumerate(MCH)]
                last_p = len(K_SEQ) - 1
                for p, ko in enumerate(K_SEQ):
                    q, j = divmod(p, 10)
                    adjr = adjp.tile([128, 2, 1280], bf, tag="adj",
                                     name=f"adjr_{e}_{ko}")
                    nc.sync.dma_start(adjr[:], adjT_r[:, ko:ko + 2, :])
                    for kk in range(2):
                        for ci, (mo, mw) in enumerate(MCH):
                            nc.tensor.matmul(
                                pss[ci], lhsT=qts[q][:, 2 * j + kk, :e],
                                rhs=adjr[:, kk, mo:mo + mw],
                                start=(p == 0 and kk == 0),
                                stop=(p == last_p and kk == 1))
                for ci, (mo, mw) in enumerate(MCH):
                    drain(0, mo, mw, pss[ci])

            # ================= Program =================

            xT_r = xT_d.ap().rearrange("(o p) m -> p o m", p=128)

            with ExitStack() as phaseA:
                pA = phaseA.enter_context(tc.tile_pool(name="pA", bufs=1))
                tra1_sb = pA.tile([128, E1 // 128, R], bf, name="tra1_sb")
                tra2_sb = pA.tile([128, E2 // 128, R], bf, name="tra2_sb")
                h1_sb = pA.tile([128, E1 // 128, R], bf, name="h1_sb")
                h2_sb = pA.tile([128, E2 // 128, R], bf, name="h2_sb")

                # -- xT resident in the slabq0 slot (exactly 2.6 MB);
                #    flat free-dim view: index kfi*1280 + m --
                xq = slabp.tile([128, 20, 512], bf, tag="slabq0", name="xq")
                xqf = xq[:].rearrange("p a b -> p (a b)")
                nc.scalar.dma_start(
                    xqf.rearrange("p (o m) -> p o m", o=NI // 128), xT_r)

                def xslice(kfi, mo, mw):
                    idx = kfi * R + mo
                    return xqf[:, idx:idx + mw]

                # -- xw1 = x @ g1w (node-major); AG chunks fired per m-block --
                with tc.tile_pool(name="wg1", bufs=1) as wp:
                    g1w_sb = load_w(wp, g1w_d, NI, E1)
                    for mt in range(MT):
                        st = stage.tile([128, E1], bf, tag="nmstage",
                                        name=f"x1st_{mt}")
                        ps = dpsum.tile([128, 512], f32, tag="dps",
                                        name=f"x1ps_{mt}")
                        for kfi in range(NI // 128):
                            nc.tensor.matmul(ps[:],
                                             lhsT=xslice(kfi, mt * 128, 128),
                                             rhs=g1w_sb[:, kfi, :],
                                             start=(kfi == 0),
                                             stop=(kfi == NI // 128 - 1))
                        nc.scalar.copy(st[:], ps[:])
                        nc.sync.dma_start(xw1_loc[mt * 128:(mt + 1) * 128, :],
                                            st[:])
                        if mt in (3, 7, 9):
                            ag_chunk(xw1_loc, xw1_ch, {3: 0, 7: 1, 9: 2}[mt])

                # -- AE enc1 (nci-inner with 4 psum banks) --
                with tc.tile_pool(name="we1", bufs=1) as wp:
                    e1w_sb = load_w(wp, e1w_d, NI, E1)
                    e1b_sb = load_b(e1b_d, E1)
                    for (mo, mw) in MCH:
                        pss = [psp.tile([128, 512], f32, tag=f"ps{n}",
                                        name=f"e1ps_{mo}_{n}")[:, :mw]
                               for n in range(4)]
                        for kfi in range(NI // 128):
                            for n in range(4):
                                nc.tensor.matmul(
                                    pss[n], lhsT=e1w_sb[:, kfi,
                                                        n * 128:(n + 1) * 128],
                                    rhs=xslice(kfi, mo, mw), start=(kfi == 0),
                                    stop=(kfi == NI // 128 - 1))
                        for n in range(4):
                            nc.scalar.activation(tra1_sb[:, n, mo:mo + mw],
                                                 pss[n], AF.Relu,
                                                 bias=e1b_sb[:, n:n + 1])

                def fm_relu_drain(out_sb):
                    def d(nci, mo, mw, ps, b):
                        nc.scalar.activation(out_sb[:, nci, mo:mo + mw], ps,
                                             AF.Relu,
                                             bias=b if b is not None else 0.0)
                    return d

                dense_fm(tra1_sb, e2w_d, e2b_d, E1, E2, fm_relu_drain(tra2_sb))

                def tra3_drain(nci, mo, mw, ps, b):
                    st = stage.tile([128, 512], bf, tag="fmstage",
                                    name=f"t3st_{nci}_{mo}")
                    nc.scalar.activation(st[:, :mw], ps, AF.Relu,
                                         bias=b if b is not None else 0.0)
                    nc.sync.dma_start(tra3_dram[:, nci, mo:mo + mw], st[:, :mw])

                dense_fm(tra2_sb, e3w_d, e3b_d, E2, E3, tra3_drain)

                # -- z' = tra3' @ zw + zb' (stream tra3' back) --
                with tc.tile_pool(name="wz", bufs=1) as wp, \
                     tc.tile_pool(name="t3s", bufs=3) as t3p:
                    zw_sb = load_w(wp, zw_d, E3, NZ)
                    zb_sb = load_b(zb_d, NZ)
                    for (mo, mw) in MCH:
                        ps = dpsum.tile([128, 512], f32, tag="dps", name=f"zps_{mo}")
                        for kfi in range(E3 // 128):
                            t3t = t3p.tile([128, 512], bf, tag="t3t",
                                           name=f"t3t_{mo}_{kfi}")
                            nc.sync.dma_start(t3t[:, :mw],
                                                tra3_dram[:, kfi, mo:mo + mw])
                            nc.tensor.matmul(ps[:NZ, :mw], lhsT=zw_sb[:, kfi, :NZ],
                                             rhs=t3t[:, :mw], start=(kfi == 0),
                                             stop=(kfi == E3 // 128 - 1))
                        nc.scalar.activation(z_sb[:, mo:mo + mw], ps[:NZ, :mw],
                                             AF.Identity, bias=zb_sb[:, 0:1])
                        zst = stage.tile([NZ, 512], f32, tag="zstage",
                                         name=f"zst_{mo}")
                        nc.scalar.activation(zst[:, :mw], ps[:NZ, :mw], AF.Identity,
                                             bias=zb_sb[:, 0:1])
                        nc.scalar.mul(zst[:, :mw], zst[:, :mw], 1.0 / TS)
                        nc.sync.dma_start(z_o[:, mo:mo + mw], zst[:, :mw])

                def h_drain(out_sb):
                    def d(gg, mo, mw, ps):
                        nc.scalar.activation(out_sb[:, gg, mo:mo + mw], ps, AF.Relu)
                    return d

                # -- spmm1 + fused u2/xw2/AG2 per m-chunk --
                with tc.tile_pool(name="wg2", bufs=1) as wp:
                    g2w_sb = load_w(wp, g2w_d, E1, E2)

                    def xw2_drain(mt, nco, ncw, ps, st):
                        nc.scalar.copy(st[:, :ncw], ps)
                        nc.sync.dma_start(
                            xw2_loc[mt * 128:(mt + 1) * 128,
                                    nco * 512:nco * 512 + ncw], st[:, :ncw])

                    def mc1_done(ci, mo, mw):
                        for kfi in range(E1 // 128):
                            nc.vector.tensor_add(
                                out=h1_sb[:, kfi, mo:mo + mw],
                                in0=h1_sb[:, kfi, mo:mo + mw],
                                in1=tra1_sb[:, kfi, mo:mo + mw])
                        dense_nm(h1_sb, g2w_sb, E1, E2, xw2_drain,
                                 mts=range(mo // 128, (mo + mw) // 128))
                        ag_chunk(xw2_loc, xw2_ch, ci)

                    spmm(src_chunked(xw1_ch), E1, [(0, E1)], h_drain(h1_sb),
                         mc_done=mc1_done)

                # -- spmm2 + fused u3/xw3/AG3 per m-chunk --
                with tc.tile_pool(name="wg3", bufs=1) as wp:
                    g3w_sb = load_w(wp, g3w_d, E2, E3)

                    def xw3_drain(mt, nco, ncw, ps, st):
                        nc.scalar.copy(st[:, :ncw], ps)
                        nc.sync.dma_start(
                            xw3_loc[mt * 128:(mt + 1) * 128,
                                    nco * 512:nco * 512 + ncw], st[:, :ncw])

                    def mc2_done(ci, mo, mw):
                        for kfi in range(E2 // 128):
                            nc.vector.tensor_add(
                                out=h2_sb[:, kfi, mo:mo + mw],
                                in0=h2_sb[:, kfi, mo:mo + mw],
                                in1=tra2_sb[:, kfi, mo:mo + mw])
                        dense_nm(h2_sb, g3w_sb, E2, E3, xw3_drain,
                                 mts=range(mo // 128, (mo + mw) // 128))
                        ag_chunk(xw3_loc, xw3_ch, ci)

                    spmm(src_chunked(xw2_ch), E2, [(0, E2)], h_drain(h2_sb),
                         mc_done=mc2_done)

            # -- spmm3 with fused u4 = relu(h3) + tra3' -> u4_dram --
            with tc.tile_pool(name="t3r", bufs=4) as t3r:
                def h3u4_drain(gg, mo, mw, ps):
                    st = stage.tile([128, 512], bf, tag="fmstage",
                                    name=f"h3st_{gg}_{mo}")
                    nc.scalar.activation(st[:, :mw], ps, AF.Relu)
                    t3 = t3r.tile([128, 512], bf, tag="t3rt",
                                  name=f"t3r_{gg}_{mo}")
                    nc.sync.dma_start(t3[:, :mw], tra3_dram[:, gg, mo:mo + mw])
                    nc.vector.tensor_add(out=st[:, :mw], in0=st[:, :mw],
                                         in1=t3[:, :mw])
                    nc.sync.dma_start(u4_dram[:, gg, mo:mo + mw], st[:, :mw])

                spmm(src_chunked(xw3_ch), E3,
                     [(0, 512), (512, 512), (1024, 512), (1536, 512)], h3u4_drain)

            # -- xw4 = u4 @ g4w -> AG --
            with ExitStack() as phaseB:
                pB = phaseB.enter_context(tc.tile_pool(name="pB", bufs=1))
                u4_sb = pB.tile([128, E3 // 128, R], bf, name="u4_sb")
                for kfi in range(E3 // 128):
                    nc.sync.dma_start(u4_sb[:, kfi, :], u4_dram[:, kfi, :])
                with tc.tile_pool(name="wg4", bufs=1) as wp:
                    g4w_sb = load_w(wp, g4w_d, E3, NZ)

                    def xw4_drain(mt, nco, ncw, ps, st):
                        nc.scalar.copy(st[:, :ncw], ps)
                        nc.sync.dma_start(xw4_loc[mt * 128:(mt + 1) * 128, :],
                                            st[:, :NZ])

                    dense_nm(u4_sb, g4w_sb, E3, NZ, xw4_drain)
                ag_single(xw4_loc, xw4_full)

            # -- spmm4: h4 (no relu) -> h4aug f32 + relu(h4) bf16 --
            def h4_drain(gg, mo, mw, ps):
                nc.vector.tensor_copy(out=h4aug[:NZ, mo:mo + mw], in_=ps)
                nc.scalar.activation(rh4_sb[:NZ, mo:mo + mw], ps, AF.Relu)

            spmm_fw(src_full(xw4_full), NZ, h4_drain)

            # -- u5 = relu(h4) + z'; xw5 -> AG (critical path first) --
            with ExitStack() as phaseC:
                pC = phaseC.enter_context(tc.tile_pool(name="pC", bufs=1))
                u5_sb = pC.tile([128, R], bf, name="u5_sb")
                nc.gpsimd.memset(u5_sb[:], 0.0)
                nc.vector.tensor_add(out=u5_sb[:NZ, :], in0=rh4_sb[:NZ, :],
                                     in1=z_sb[:])
                with tc.tile_pool(name="wg5", bufs=1) as wp:
                    g5w_sb = wp.tile([128, K], bf, name="g5w_sb")
                    nc.scalar.dma_start(g5w_sb[:], g5wp_d[:])

                    def xw5_drain(mt, nco, ncw, ps, st):
                        nc.scalar.copy(st[:, :ncw], ps)
                        nc.sync.dma_start(xw5_loc[mt * 128:(mt + 1) * 128, :],
                                            st[:, :K])

                    dense_nm(u5_sb, g5w_sb, 128, K, xw5_drain)
                ag_single(xw5_loc, xw5_full)

            # -- x_bar = relu(relu(h4) @ fc1_w + b) (fills AG5 gap) --
            with tc.tile_pool(name="wfc", bufs=1) as wp:
                fc1_sb = wp.tile([128, NI], bf, name="fc1_sb")
                nc.scalar.dma_start(fc1_sb[:], fc1aug_d[:])

                def fc1_drain(mt, nco, ncw, ps, st):
                    nc.scalar.activation(st[:, :ncw], ps, AF.Relu)
                    nc.sync.dma_start(
                        xbar_o[mt * 128:(mt + 1) * 128,
                               nco * 512:nco * 512 + ncw], st[:, :ncw])

                dense_nm(rh4_sb, fc1_sb, 128, NI, fc1_drain, out_dt=f32)

            # -- q head (depends only on h4; also fills AG5 gap) --
            nc.vector.tensor_mul(out=h4sq[:], in0=h4aug[:], in1=h4aug[:])
            with tc.tile_pool(name="hq", bufs=2, space="PSUM") as hq:
                for mt in range(MT):
                    ms = slice(mt * 128, (mt + 1) * 128)
                    psq = hq.tile([128, K], f32, tag="psq", name=f"psq_{mt}")
                    nc.tensor.matmul(psq[:], lhsT=h4aug[:, ms], rhs=qmat_sb[:],
                                     start=True, stop=False)
                    nc.tensor.matmul(psq[:], lhsT=h4sq[:, ms], rhs=ones_sb[:],
                                     start=False, stop=True)
                    tq = small.tile([128, K], f32, tag="tq", name=f"tq_{mt}")
                    nc.scalar.add(tq[:], psq[:], 1.0)
                    qn = small.tile([128, K], f32, tag="qn", name=f"qn_{mt}")
                    nc.vector.reciprocal(qn[:], tq[:])
                    s1 = small.tile([128, 1], f32, tag="s1", name=f"s1_{mt}")
                    nc.vector.reduce_sum(out=s1[:], in_=qn[:], axis=AX.X)
                    nc.vector.reciprocal(s1[:], s1[:])
                    qv = small.tile([128, K], f32, tag="qv", name=f"qv_{mt}")
                    nc.vector.tensor_scalar_mul(qv[:], qn[:], s1[:])
                    nc.sync.dma_start(q_o[ms, :], qv[:])

            # -- spmm5 with fused predict softmax per m-chunk --
            with tc.tile_pool(name="hp", bufs=2, space="PSUM") as hp:
                def h5_drain(gg, mo, mw, ps):
                    nc.vector.tensor_copy(out=h5_sb[:, mo:mo + mw], in_=ps)
                    for mt in range(mo // 128, (mo + mw) // 128):
                        ms = slice(mt * 128, (mt + 1) * 128)
                        pst = hp.tile([128, K], f32, tag="pst", name=f"pst_{mt}")
                        nc.tensor.transpose(pst[:], h5_sb[:, ms], ident_sb[:])
                        mx = small.tile([128, 1], f32, tag="mx", name=f"mx_{mt}")
                        nc.vector.reduce_max(out=mx[:], in_=pst[:], axis=AX.X)
                        nc.vector.tensor_scalar_mul(mx[:], mx[:], -1.0)
                        ev = small.tile([128, K], f32, tag="ev", name=f"ev_{mt}")
                        nc.scalar.activation(ev[:], pst[:], AF.Exp, bias=mx[:])
                        s2 = small.tile([128, 1], f32, tag="s2", name=f"s2_{mt}")
                        nc.vector.reduce_sum(out=s2[:], in_=ev[:], axis=AX.X)
                        nc.vector.reciprocal(s2[:], s2[:])
                        pv = small.tile([128, K], f32, tag="pv", name=f"pv_{mt}")
                        nc.vector.tensor_scalar_mul(pv[:], ev[:], s2[:])
                        nc.sync.dma_start(pred_o[ms, :], pv[:])

                spmm_fw(src_full(xw5_full), K, h5_drain)

    nc.compile()
    return nc


def _prep_inputs(inputs):
    """Shard + lay out full inputs for the 8 cores."""
    f32 = np.float32
    x = np.asarray(inputs["x"], f32)
    adj = np.asarray(inputs["adj"], f32)

    def b16(a):
        return np.ascontiguousarray(np.asarray(a, f32).astype(BF16))

    def col(a):
        return np.ascontiguousarray(np.asarray(a, f32).reshape(-1, 1))

    shared = {
        "e1w": b16(np.asarray(inputs["ae_enc1_w"], f32) * TS),
        "e2w": b16(inputs["ae_enc2_w"]),
        "e3w": b16(inputs["ae_enc3_w"]),
        "zw": b16(inputs["ae_z_w"]),
        "e1b": col(np.asarray(inputs["ae_enc1_b"], f32) * TS),
        "e2b": col(np.asarray(inputs["ae_enc2_b"], f32) * TS),
        "e3b": col(np.asarray(inputs["ae_enc3_b"], f32) * TS),
        "zb": col(np.asarray(inputs["ae_z_b"], f32) * TS),
        "g1w": b16(inputs["gnn1_w"]),
        "g2w": b16(np.asarray(inputs["gnn2_w"], f32) * (1.0 - SIGMA)),
        "g3w": b16(np.asarray(inputs["gnn3_w"], f32) * (1.0 - SIGMA)),
        "g4w": b16(np.asarray(inputs["gnn4_w"], f32) * (1.0 - SIGMA)),
    }
    g5 = np.zeros((128, K), f32)
    g5[:NZ] = np.asarray(inputs["gnn5_w"], f32) * (1.0 - SIGMA)
    shared["g5wp"] = b16(g5)
    fca = np.zeros((128, NI), f32)
    fca[:NZ] = np.asarray(inputs["fc1_w"], f32)
    fca[NZ] = np.asarray(inputs["fc1_b"], f32)
    shared["fc1aug"] = b16(fca)
    cl = np.asarray(inputs["cluster"], f32)  # [K, NZ]
    qm = np.zeros((128, K), f32)
    qm[:NZ] = -2.0 * cl.T
    qm[NZ] = (cl * cl).sum(axis=1)
    shared["qmat"] = np.ascontiguousarray(qm)

    in_maps = []
    for c in range(NCORES):
        r0 = c * R
        r1 = min(N, r0 + R)
        nreal = max(0, r1 - r0)
        xT = np.zeros((NI, R), BF16)
        if nreal > 0:
            xT[:, :nreal] = x[r0:r1].T.astype(BF16)
        adjT = np.zeros((NP, R), BF16)
        if nreal > 0:
            adjT[:N, :nreal] = adj[r0:r1].T.astype(BF16)
        m = dict(shared)
        m["xT"] = xT
        m["adjT"] = adjT
        in_maps.append(m)
    return in_maps


def kernel(**inputs):
    global _cached_nc
    from concourse.bass_utils import run_bass_kernel_spmd

    in_maps = _prep_inputs(inputs)
    if _cached_nc is None:
        _cached_nc = _build()
    res = run_bass_kernel_spmd(_cached_nc, in_maps, core_ids=list(range(NCORES)))
    outs = res.results
    x_bar = np.concatenate([o["xbar_o"] for o in outs], axis=0)[:N]
    q = np.concatenate([o["q_o"] for o in outs], axis=0)[:N]
    predict = np.concatenate([o["pred_o"] for o in outs], axis=0)[:N]
    z = np.concatenate([o["z_o"] for o in outs], axis=1).T[:N]
    z = np.ascontiguousarray(z)
    return (x_bar, q, predict, z)
